# revision 1
# baseline (speedup 1.0000x reference)
"""DGRec kernel for 8 TRN2 NeuronCores (Bass/Tile).

Strategy:
  - Host: index-only prep + table row-selection sharding. Live-session pruning
    (sessions never referenced by src0/idx0/cur_sidx are dead), sessions sorted
    by lens desc and dealt round-robin so every core shares one static
    shrinking-prefix LSTM schedule. Per-core compact item vocab; per-core user
    rows in session order; item vocab shard for logits.
  - Device: renorm tables (bf16), dma_gather transpose-mode loads tokens
    directly into [D, sessions] column layout, LSTM gates via 4-slot PSUM
    matmuls + ACT sigmoid/tanh (per-gate bias), bf16 DVE state updates,
    GAT via one-hot matmuls (edge softmax, segment sums), two AllGathers,
    vocab-sharded logits matmul.
"""
import sys
sys.path.insert(0, '/opt/trn_rl_repo')

import numpy as np


class _PhaseDone(Exception):
    pass


NCORES = 8
D = 128
T = 20
N0 = 25600
N1 = 2560
N2 = 512
E1 = 5120
NI = 50000
CURB = 128          # cur block rows per core (slot CURB-1 reserved zero row)
DST0_PER_CORE = N1 // NCORES  # 320
B1 = 384            # feat1 block rows per core (320 padded to 384)


def _rup(x, m):
    return (int(x) + m - 1) // m * m


def _wrap16(idx):
    """[n] int -> [128, n//16] int16: idx i at [i%16, i//16], replicated x8."""
    idx = np.asarray(idx, np.int16)
    n = len(idx)
    assert n % 16 == 0
    a = idx.reshape(n // 16, 16).T  # [16, n//16]
    return np.tile(a, (8, 1))


def host_prep(inputs):
    lens = np.asarray(inputs['lens']).astype(np.int64)
    seqs = np.asarray(inputs['padded_seqs']).astype(np.int64)
    uids = np.asarray(inputs['uids']).astype(np.int64)
    cur_sidx = np.asarray(inputs['cur_sidx']).astype(np.int64)
    src0 = np.asarray(inputs['src0']).astype(np.int64)
    dst0 = np.asarray(inputs['dst0']).astype(np.int64)
    idx0 = np.asarray(inputs['idx0']).astype(np.int64)
    src1 = np.asarray(inputs['src1']).astype(np.int64)
    dst1 = np.asarray(inputs['dst1']).astype(np.int64)
    idx1 = np.asarray(inputs['idx1']).astype(np.int64)
    user_emb = np.ascontiguousarray(np.asarray(inputs['user_emb'], np.float32))
    item_emb = np.ascontiguousarray(np.asarray(inputs['item_emb'], np.float32))

    # ---- live sessions, sorted by len desc, per-len-group padded to mult 8
    live_mask = np.zeros(N0, bool)
    live_mask[src0] = True
    live_mask[idx0] = True
    live_mask[cur_sidx] = True
    live = np.where(live_mask)[0]
    order = live[np.argsort(-lens[live], kind='stable')]
    lens_live = lens[order]
    parts = []
    grp_ceil = {}
    for L in range(T, 0, -1):
        grp = order[lens_live == L]
        pad = (-len(grp)) % NCORES
        parts.append(grp)
        grp_ceil[L] = (len(grp) + pad) // NCORES
        if pad:
            parts.append(np.full(pad, -1, np.int64))
    order_p = np.concatenate(parts)
    percore = len(order_p) // NCORES
    NL = _rup(percore, 128)
    extra = NL * NCORES - len(order_p)
    order_p = np.concatenate([order_p, np.full(extra, -1, np.int64)])
    core_sessions = [order_p[k::NCORES] for k in range(NCORES)]
    act = [sum(grp_ceil[L] for L in range(t + 1, T + 1)) for t in range(T)]
    act = [min(a, NL) for a in act]

    # ---- FTAB layout: per-core block [NL feat rows][CURB cur rows]
    BLK = NL + CURB
    ZROW = NL + CURB - 1  # core 0 block, slot CURB-1: reserved all-zero row
    sess2pos = np.full(N0, -1, np.int64)
    core_local = []       # per core: {session: local index}
    JU_ = NL // 128
    for k in range(NCORES):
        sess = core_sessions[k]
        real = sess >= 0
        li = np.where(real)[0]
        sess2pos[sess[real]] = k * BLK + (li % 128) * JU_ + li // 128
        core_local.append({int(s): i for i, s in enumerate(sess) if s >= 0})
    # cur block: per core, unique local cur sessions -> slots 0..cnt-1
    cur_pos = {}
    cur_slot_local = [[] for _ in range(NCORES)]
    for s in np.unique(cur_sidx):
        owner = -1
        for k in range(NCORES):
            if int(s) in core_local[k]:
                owner = k
                break
        assert owner >= 0
        slot = len(cur_slot_local[owner])
        assert slot < 112
        cur_pos[int(s)] = owner * BLK + NL + slot
        cur_slot_local[owner].append(core_local[owner][int(s)])
    # remap: sessions in cur_sidx read hn (feat.at[cur].set(cur))
    for s in np.unique(cur_sidx):
        sess2pos[s] = cur_pos[int(s)]

    # ---- per-core compact vocab + token idx per step (pass 1: sizes)
    vocab_sizes = []
    for k in range(NCORES):
        sess = core_sessions[k]
        smax = np.maximum(sess, 0)
        toks = np.where(sess[None, :] >= 0, seqs[smax].T, 0)
        used = np.concatenate([toks[t, :act[t]] for t in range(T) if act[t] > 0])
        vocab_sizes.append(len(np.unique(np.concatenate([[0], used]))))
    VC0 = _rup(max(vocab_sizes), 128)

    core_vocab = []
    core_tok = []   # [T] arrays of compact ids, each length rup(act[t],128)
    for k in range(NCORES):
        sess = core_sessions[k]
        smax = np.maximum(sess, 0)
        toks = np.where(sess[None, :] >= 0, seqs[smax].T, 0)  # [T, NL]
        used = np.concatenate([toks[t, :act[t]] for t in range(T) if act[t] > 0])
        vocab = np.unique(np.concatenate([[0], used]))
        lut = np.full(NI + 1, -1, np.int64)
        # table rows are written contiguously from [128, J, 128] staging:
        # staging slot (p, j) = vocab[128*j+p] lands at DRAM row p*J + j
        JI_ = VC0 // 128
        ar = np.arange(len(vocab))
        lut[vocab] = (ar % 128) * JI_ + ar // 128
        per_step = []
        for t in range(T):
            a128 = _rup(max(act[t], 1), 128)
            ct = np.zeros(a128, np.int64)
            ct[:act[t]] = lut[toks[t, :act[t]]]
            per_step.append(ct)
        core_vocab.append(vocab)
        core_tok.append(per_step)
    VC = VC0
    assert VC < 32000

    # ---- GAT0: edges sharded by dst range, sorted by dst
    g0 = []
    for k in range(NCORES):
        lo, hi = k * DST0_PER_CORE, (k + 1) * DST0_PER_CORE
        e = np.where((dst0 >= lo) & (dst0 < hi))[0]
        e = e[np.argsort(dst0[e], kind='stable')]
        g0.append(e)
    E0C = _rup(max(len(e) for e in g0), 128)
    NCH0 = E0C // 128

    # ---- GAT1 (redundant on all cores): edges sorted by dst
    e1 = np.argsort(dst1, kind='stable')
    assert E1 % 128 == 0
    NCH1 = E1 // 128

    def f1pos(node):
        node = np.asarray(node)
        loc = node % DST0_PER_CORE
        return (node // DST0_PER_CORE) * B1 + (loc % 128) * 3 + loc // 128

    g1_pay = f1pos(src1[e1])
    g1_fd = f1pos(idx1[dst1[e1]])
    g1_dst = dst1[e1]
    g1_res = f1pos(idx1)  # [512]

    cur_idx = np.array([cur_pos[int(s)] for s in cur_sidx], np.int64)

    LSH = NI // NCORES          # 6250
    LSHP = _rup(LSH, 128)       # 6272

    meta = dict(NL=NL, BLK=BLK, VC=VC, E0C=E0C, NCH0=NCH0, NCH1=NCH1,
                LSH=LSH, LSHP=LSHP, act=act, ZROW=ZROW)

    # ---- IDX16 buffer layout (columns of 16-wrapped idx); same offsets all cores
    seg_off = {}
    _w = [0]

    def add_seg(name, n):
        seg_off[name] = _w[0]
        _w[0] += n // 16

    for t in range(T):
        add_seg(f'tok{t}', _rup(max(act[t], 1), 128))
    add_seg('g0pay', E0C)
    add_seg('g0fd', E0C)
    add_seg('g0res', B1)
    add_seg('g1pay', E1)
    add_seg('g1fd', E1)
    add_seg('g1res', 512)
    add_seg('cur', 512)
    add_seg('lsh', LSHP)
    add_seg('hnrow', 112)
    W16 = _w[0]
    meta['seg_off'] = seg_off
    meta['W16'] = W16

    in_maps = []
    for k in range(NCORES):
        sess = core_sessions[k]
        smax = np.maximum(sess, 0)
        vocab = core_vocab[k]

        idx16 = np.zeros((128, W16), np.int16)

        def put(name, vals):
            v = np.asarray(vals, np.int64)
            assert v.min() >= -1 and v.max() < 32767, (name, v.min(), v.max())
            o = seg_off[name]
            w = _wrap16(v.astype(np.int16))
            idx16[:, o:o + w.shape[1]] = w

        for t in range(T):
            put(f'tok{t}', core_tok[k][t])
        e = g0[k]
        pay = np.full(E0C, ZROW, np.int64)
        fde = np.full(E0C, ZROW, np.int64)
        dstl = np.full(E0C, -1, np.int64)
        pay[:len(e)] = sess2pos[src0[e]]
        fde[:len(e)] = sess2pos[idx0[dst0[e]]]
        dstl[:len(e)] = dst0[e] - k * DST0_PER_CORE
        put('g0pay', pay)
        put('g0fd', fde)
        res0 = np.full(B1, ZROW, np.int64)
        res0[:DST0_PER_CORE] = sess2pos[
            idx0[k * DST0_PER_CORE:(k + 1) * DST0_PER_CORE]]
        put('g0res', res0)
        put('g1pay', g1_pay)
        put('g1fd', g1_fd)
        put('g1res', g1_res)
        put('cur', cur_idx)
        lshi = np.zeros(LSHP, np.int64)
        ar = np.arange(LSH)
        lshi[:LSH] = (ar % 128) * (LSHP // 128) + ar // 128
        put('lsh', lshi)
        hnrow = np.zeros(112, np.int64)
        cs = np.asarray(cur_slot_local[k], np.int64)
        assert len(cs) <= 112
        if len(cs):
            hnrow[:len(cs)] = (cs % 128) * (NL // 128) + cs // 128
        put('hnrow', hnrow)

        # dst-local values for one-hot compare: edge (chunk c, partition p)
        dstl_all = np.full((128, NCH0 + NCH1), -1, np.int32)
        dstl_all[:, :NCH0] = dstl.reshape(NCH0, 128).T
        dstl_all[:, NCH0:] = g1_dst.reshape(NCH1, 128).T

        item_sub = np.zeros((VC, D), np.float32)
        item_sub[:len(vocab)] = item_emb[vocab]
        usr = np.zeros((NL, D), np.float32)
        usr[:] = user_emb[uids[smax]]
        usr[sess < 0] = 0.0
        ishard = np.zeros((LSHP, D), np.float32)
        ishard[:LSH] = item_emb[1 + k * LSH: 1 + (k + 1) * LSH]

        in_maps.append({
            'item_sub': item_sub,
            'usr': usr,
            'ishard': ishard,
            'WihT': np.ascontiguousarray(np.asarray(inputs['Wih'], np.float32).T),
            'WhhT': np.ascontiguousarray(np.asarray(inputs['Whh'], np.float32).T),
            'bih': np.asarray(inputs['bih'], np.float32),
            'bhh': np.asarray(inputs['bhh'], np.float32),
            'W1T': np.ascontiguousarray(np.asarray(inputs['W1'], np.float32).T),
            'W2T': np.ascontiguousarray(np.asarray(inputs['W2'], np.float32).T),
            'gW0T': np.ascontiguousarray(np.asarray(inputs['gW0'], np.float32).T),
            'gW1T': np.ascontiguousarray(np.asarray(inputs['gW1'], np.float32).T),
            'gb0': np.asarray(inputs['gb0'], np.float32),
            'gb1': np.asarray(inputs['gb1'], np.float32),
            'idx16': idx16,
            'dstl': dstl_all,
        })
    return in_maps, meta


# ============================ device program ============================

def build_program(meta):
    import os
    PHASE = int(os.environ.get('KPHASE', '9'))
    KSUB = int(os.environ.get('KSUB', '9'))
    KT = int(os.environ.get('KT', '99'))
    KCH = int(os.environ.get('KCH', '9'))
    import contextlib
    import concourse.bass as bass
    import concourse.mybir as mybir
    import concourse.tile as tile
    from concourse import bacc
    from concourse.masks import make_identity

    NL = meta['NL']
    BLK = meta['BLK']
    VC = meta['VC']
    NCH0 = meta['NCH0']
    NCH1 = meta['NCH1']
    LSH = meta['LSH']
    LSHP = meta['LSHP']
    act = meta['act']
    seg = meta['seg_off']
    W16 = meta['W16']
    JI = VC // 128
    JU = NL // 128
    JL = LSHP // 128
    FT = mybir.dt.float32
    BF = mybir.dt.bfloat16
    AF = mybir.ActivationFunctionType
    OP = mybir.AluOpType

    nc = bacc.Bacc("TRN2", target_bir_lowering=False, debug=False,
                   num_devices=NCORES)

    def param(name, shape, dt=FT):
        return nc.declare_dram_parameter(name, list(shape), dt, isOutput=False)

    item_sub = param('item_sub', [VC, D])
    usr = param('usr', [NL, D])
    ishard = param('ishard', [LSHP, D])
    WihT = param('WihT', [D, 512])
    WhhT = param('WhhT', [D, 512])
    bih = param('bih', [512])
    bhh = param('bhh', [512])
    W1T = param('W1T', [256, D])
    W2T = param('W2T', [256, D])
    gW0T = param('gW0T', [D, D])
    gW1T = param('gW1T', [D, D])
    gb0 = param('gb0', [D])
    gb1 = param('gb1', [D])
    idx16_p = param('idx16', [128, W16], mybir.dt.int16)
    dstl_p = param('dstl', [128, NCH0 + NCH1], mybir.dt.int32)
    out_p = nc.declare_dram_parameter('out', [N2, LSH], FT, isOutput=True)

    def rows_ap(handle_ap, j_count, base_elem=0):
        """view rows [128*j_count, D] of a DRAM tensor as [128, j, D], row=128j+p"""
        t = handle_ap if isinstance(handle_ap, bass.AP) else handle_ap[:]
        return bass.AP(tensor=t.tensor, offset=t.offset + base_elem,
                       ap=[[D, 128], [128 * D, j_count], [1, D]])

    def cont_ap(handle_ap, j_count, base_elem=0):
        """contiguous [128, j, D] view: slot (p, j) -> DRAM row p*j_count + j"""
        t = handle_ap if isinstance(handle_ap, bass.AP) else handle_ap[:]
        return bass.AP(tensor=t.tensor, offset=t.offset + base_elem,
                       ap=[[j_count * D, 128], [D, j_count], [1, D]])

    with tile.TileContext(nc) as tc:
        try:
            ctx = contextlib.ExitStack()
            ctx.__enter__()
            glob = ctx.enter_context(tc.tile_pool(name='glob', bufs=1))
            dram = ctx.enter_context(tc.tile_pool(name='dram', bufs=1,
                                                  space='DRAM'))

            ITBL = dram.tile([VC, D], BF)
            LSHARD = dram.tile([LSHP, D], BF)
            HNROWS = dram.tile([NL, D], BF)
            AGIN = dram.tile([BLK, D], BF)
            FTAB = dram.tile([NCORES * BLK, D], BF, addr_space='Shared')
            AG2IN = dram.tile([B1, D], BF)
            F1TAB = dram.tile([NCORES * B1, D], BF, addr_space='Shared')

            # ---- global constants / index tiles
            idx_sb = glob.tile([128, W16], mybir.dt.int16)
            nc.sync.dma_start(idx_sb[:], idx16_p[:])

            def seg_ap(name, n, off=0):
                o = seg[name] + off // 16
                return idx_sb[:, o:o + n // 16]

            GMAX = 512

            def gather_t(out_full, colbase, tab, name, n, queue=0):
                """transpose-mode gather of n idx (mult 128) from segment
                `name` into out_full[:, 0, colbase:colbase+n], split <=GMAX"""
                for o in range(0, n, GMAX):
                    w = min(GMAX, n - o)
                    nc.gpsimd.dma_gather(
                        out_ap=out_full[:, :, colbase + o:colbase + o + w],
                        in_ap=tab[:], idxs_ap=seg_ap(name, w, o),
                        num_idxs=w, num_idxs_reg=w, elem_size=D,
                        transpose=True, queue_num=queue)

            def gather_rows(out_tile, tab, name, n, queue=0):
                """non-transpose gather of n idx into [128, n//128, 128]"""
                for o in range(0, n, GMAX):
                    w = min(GMAX, n - o)
                    nc.gpsimd.dma_gather(
                        out_ap=out_tile[:, o // 128:(o + w) // 128, :],
                        in_ap=tab[:], idxs_ap=seg_ap(name, w, o),
                        num_idxs=w, num_idxs_reg=w, elem_size=D,
                        transpose=False, queue_num=queue)

            ident = glob.tile([128, 128], BF)
            make_identity(nc, ident[:])
            iota_i = glob.tile([128, 512], mybir.dt.int32)
            nc.gpsimd.iota(iota_i[:], pattern=[[1, 512]], base=0,
                           channel_multiplier=0)
            iotaf = glob.tile([128, 512], FT)
            nc.vector.tensor_copy(iotaf[:], iota_i[:])
            dstl_i = glob.tile([128, NCH0 + NCH1], mybir.dt.int32)
            nc.sync.dma_start(dstl_i[:], dstl_p[:])
            dstf = glob.tile([128, NCH0 + NCH1], FT)
            nc.vector.tensor_copy(dstf[:], dstl_i[:])
            ones1 = glob.tile([1, 128], FT)
            nc.vector.memset(ones1[:], 1.0)

            # ---- weights (cast to bf16 via gpsimd DMA)
            wih16 = glob.tile([128, 512], BF)
            nc.gpsimd.dma_start(wih16[:], WihT[:])
            whh16 = glob.tile([128, 512], BF)
            nc.gpsimd.dma_start(whh16[:], WhhT[:])
            w1_16 = glob.tile([128, 2, 128], BF)
            nc.gpsimd.dma_start(w1_16[:], rows_ap(W1T, 2))
            w2_16 = glob.tile([128, 2, 128], BF)
            nc.gpsimd.dma_start(w2_16[:], rows_ap(W2T, 2))
            gw0_16 = glob.tile([128, 128], BF)
            nc.gpsimd.dma_start(gw0_16[:], gW0T[:])
            gw1_16 = glob.tile([128, 128], BF)
            nc.gpsimd.dma_start(gw1_16[:], gW1T[:])
            gb0_sb = glob.tile([128, 1], FT)
            nc.sync.dma_start(gb0_sb[:], bass.AP(tensor=gb0, offset=0,
                                                 ap=[[1, 128], [1, 1]]))
            gb1_sb = glob.tile([128, 1], FT)
            nc.sync.dma_start(gb1_sb[:], bass.AP(tensor=gb1, offset=0,
                                                 ap=[[1, 128], [1, 1]]))
            bi_sb = glob.tile([128, 4], FT)
            nc.sync.dma_start(bi_sb[:], bass.AP(tensor=bih, offset=0,
                                                ap=[[1, 128], [128, 4]]))
            bh_sb = glob.tile([128, 4], FT)
            nc.sync.dma_start(bh_sb[:], bass.AP(tensor=bhh, offset=0,
                                                ap=[[1, 128], [128, 4]]))
            bias = glob.tile([128, 4], FT)
            nc.vector.tensor_add(bias[:], bi_sb[:], bh_sb[:])

            # ---- renorm: rows of src -> unit-clipped bf16 [128, J, 128]
            def renorm(pool, src, J, zero_row0=False, rows=None, sub=False):
                stg = pool.tile([128, J, 128], FT, tag='rn_stg')
                if rows is None or rows == 128 * J:
                    nc.sync.dma_start(stg[:], rows_ap(src, J))
                else:
                    jf = rows // 128
                    rem = rows - jf * 128
                    if jf:
                        nc.sync.dma_start(stg[:, :jf, :], rows_ap(src, jf))
                    if rem:
                        nc.vector.memset(stg[:, jf, :], 0.0)
                        s = src[:]
                        nc.sync.dma_start(
                            stg[:rem, jf, :],
                            bass.AP(tensor=s.tensor,
                                    offset=s.offset + jf * 128 * D,
                                    ap=[[D, rem], [1, D]]))
                if sub and KSUB < 3:
                    raise _PhaseDone()
                sumsq = pool.tile([128, J], FT, tag='rn_sumsq')
                sq16 = pool.tile([128, J, 128], BF, tag='rn_sq')
                nc.scalar.activation(out=sq16[:], in_=stg[:], func=AF.Square)
                nc.vector.tensor_reduce(out=sumsq[:], in_=sq16[:],
                                        axis=mybir.AxisListType.X, op=OP.add)
                if sub and KSUB < 4:
                    raise _PhaseDone()
                nrm = pool.tile([128, J], FT, tag='rn_nrm')
                nc.scalar.activation(out=nrm[:], in_=sumsq[:], func=AF.Sqrt)
                nc.vector.tensor_scalar_max(nrm[:], nrm[:], 1e-12)
                rcp = pool.tile([128, J], FT, tag='rn_rcp')
                nc.vector.reciprocal(rcp[:], nrm[:])
                nc.vector.tensor_scalar_min(rcp[:], rcp[:], 1.0)
                if sub and KSUB < 5:
                    raise _PhaseDone()
                out16 = pool.tile([128, J, 128], BF, tag='rn_out')
                J0 = (3 * J // 4) if J > 4 else J
                if J0:
                    r = rcp[:]
                    bc = bass.AP(tensor=r.tensor, offset=r.offset,
                                 ap=[r.ap[0], [r.ap[1][0], J0], [0, 128]])
                    nc.vector.tensor_tensor(out=out16[:, :J0, :],
                                            in0=stg[:, :J0, :], in1=bc,
                                            op=OP.mult)
                for j in range(J0, J):
                    nc.scalar.activation(out=out16[:, j, :], in_=stg[:, j, :],
                                         func=AF.Copy, scale=rcp[:, j:j + 1])
                if zero_row0:
                    nc.vector.memset(out16[0:1, 0, :], 0.0)
                return out16

            # ================= precompute: item table =================
            if KSUB < 2:
                raise _PhaseDone()
            with tc.tile_pool(name='pre_i', bufs=1) as pp:
                it16 = renorm(pp, item_sub, JI, zero_row0=True, sub=True)
                if KSUB < 6:
                    raise _PhaseDone()
                nc.sync.dma_start(cont_ap(ITBL, JI), it16[:])

            # ================= LSTM =================
            if PHASE < 2:
                raise _PhaseDone()
            hT = glob.tile([128, NL], BF)
            cT = glob.tile([128, NL], BF)
            nc.vector.memset(hT[:], 0.0)
            nc.vector.memset(cT[:], 0.0)

            A0 = _rup(act[0], 128)
            with (
                tc.tile_pool(name='lstm_x', bufs=3) as xp,
                tc.tile_pool(name='lstm_g', bufs=3) as sp,
                tc.tile_pool(name='lstm_ps', bufs=2, space='PSUM') as gp,
            ):
                for t in range(T):
                    if t >= KT:
                        break
                    a = act[t]
                    if a == 0:
                        break
                    a128 = _rup(a, 128)
                    xT = xp.tile([128, 1, A0], BF, tag='xT')
                    gather_t(xT, 0, ITBL, f'tok{t}', a128)
                    if KCH < 1:
                        continue
                    nch = (a + 511) // 512
                    for c in range(nch):
                        cs = c * 512
                        cw = min(512, a - cs)
                        ce = cs + cw
                        g4 = gp.tile([128, 4, 512], FT, tag='g4')
                        for g in range(4):
                            nc.tensor.matmul(
                                g4[:, g, :cw],
                                lhsT=wih16[:, g * 128:(g + 1) * 128],
                                rhs=xT[:, 0, cs:ce], start=True, stop=(t == 0))
                            if t > 0:
                                nc.tensor.matmul(
                                    g4[:, g, :cw],
                                    lhsT=whh16[:, g * 128:(g + 1) * 128],
                                    rhs=hT[:, cs:ce], start=False, stop=True)
                        if KCH < 2:
                            continue
                        sg = sp.tile([128, 4, 512], BF, tag='sg')
                        for g, fn in ((0, AF.Sigmoid), (1, AF.Sigmoid),
                                      (2, AF.Tanh), (3, AF.Sigmoid)):
                            nc.scalar.activation(out=sg[:, g, :cw],
                                                 in_=g4[:, g, :cw], func=fn,
                                                 bias=bias[:, g:g + 1])
                        if KCH < 3:
                            continue
                        if t > 0:
                            tmp = sp.tile([128, 512], BF, tag='tmp')
                            nc.vector.tensor_mul(tmp[:, :cw], sg[:, 0, :cw],
                                                 sg[:, 2, :cw])
                            nc.vector.tensor_mul(cT[:, cs:ce], cT[:, cs:ce],
                                                 sg[:, 1, :cw])
                            nc.vector.tensor_add(cT[:, cs:ce], cT[:, cs:ce],
                                                 tmp[:, :cw])
                        else:
                            nc.vector.tensor_mul(cT[:, cs:ce], sg[:, 0, :cw],
                                                 sg[:, 2, :cw])
                        th = sp.tile([128, 512], BF, tag='th')
                        nc.scalar.activation(out=th[:, :cw], in_=cT[:, cs:ce],
                                             func=AF.Tanh)
                        nc.vector.tensor_mul(hT[:, cs:ce], sg[:, 3, :cw],
                                             th[:, :cw])

            # ============ user renorm + feat + transposes + AG1 ============
            if PHASE < 3:
                raise _PhaseDone()
            with (
                tc.tile_pool(name='feat', bufs=1) as fp,
                tc.tile_pool(name='feat_ps', bufs=2, space='PSUM') as fps,
                tc.tile_pool(name='tp_ps', bufs=2, space='PSUM') as tps,
            ):
                u16 = renorm(fp, usr, JU)
                longT = fp.tile([128, NL], BF)
                for j in range(JU):
                    pt = tps.tile([128, 128], BF, tag='tp')
                    nc.tensor.transpose(pt[:], u16[:, j, :], ident[:])
                    nc.vector.tensor_copy(longT[:, j * 128:(j + 1) * 128],
                                          pt[:])

                featT = fp.tile([128, NL], BF)
                for c in range((NL + 511) // 512):
                    cs = c * 512
                    cw = min(512, NL - cs)
                    ps = fps.tile([128, 512], FT, tag='fps')
                    nc.tensor.matmul(ps[:, :cw], lhsT=w1_16[:, 0, :],
                                     rhs=longT[:, cs:cs + cw], start=True,
                                     stop=False)
                    nc.tensor.matmul(ps[:, :cw], lhsT=w1_16[:, 1, :],
                                     rhs=hT[:, cs:cs + cw], start=False,
                                     stop=True)
                    nc.scalar.activation(out=featT[:, cs:cs + cw],
                                         in_=ps[:, :cw], func=AF.Relu)

                fr = fp.tile([128, JU, 128], BF)
                hr = fp.tile([128, JU, 128], BF)
                for j in range(JU):
                    pt = tps.tile([128, 128], BF, tag='tp')
                    nc.tensor.transpose(pt[:], featT[:, j * 128:(j + 1) * 128],
                                        ident[:])
                    nc.vector.tensor_copy(fr[:, j, :], pt[:])
                    pt2 = tps.tile([128, 128], BF, tag='tp')
                    nc.tensor.transpose(pt2[:], hT[:, j * 128:(j + 1) * 128],
                                        ident[:])
                    nc.vector.tensor_copy(hr[:, j, :], pt2[:])
                nc.sync.dma_start(cont_ap(HNROWS, JU), hr[:])
                nc.sync.dma_start(cont_ap(AGIN, JU), fr[:])
                curs = fp.tile([128, 1, 128], BF)
                nc.vector.memset(curs[:], 0.0)
                nc.gpsimd.dma_gather(
                    out_ap=curs[:], in_ap=HNROWS[:],
                    idxs_ap=seg_ap('hnrow', 112),
                    num_idxs=112, num_idxs_reg=112, elem_size=D,
                    transpose=False, queue_num=0)
                ag = AGIN[:]
                nc.sync.dma_start(
                    bass.AP(tensor=ag.tensor, offset=ag.offset + NL * D,
                            ap=[[D, 128], [1, D]]),
                    curs[:, 0, :])
                nc.gpsimd.collective_compute(
                    'AllGather', OP.bypass,
                    replica_groups=[list(range(NCORES))],
                    ins=[AGIN.opt()], outs=[FTAB.opt()])

            # ================= GAT layers =================
            def gat_layer(pool, pps, tab, pay_seg, fd_seg, nch, dst_off, ndst,
                          gw16, gb_sb, res_seg, res_n):
                E = nch * 128
                pay = pool.tile([128, nch, 128], BF, tag='pay')
                gather_rows(pay, tab, pay_seg, E)
                fde = pool.tile([128, nch, 128], BF, tag='fde')
                gather_rows(fde, tab, fd_seg, E)
                score = pool.tile([128, nch], FT, tag='score')
                prod = pool.tile([128, nch, 128], BF, tag='prod')
                nc.vector.tensor_mul(prod[:], pay[:], fde[:])
                nc.vector.tensor_reduce(out=score[:], in_=prod[:],
                                        axis=mybir.AxisListType.X, op=OP.add)
                w = pool.tile([128, nch], FT, tag='w')
                nc.scalar.activation(out=w[:], in_=score[:], func=AF.Exp)
                w16 = pool.tile([128, nch], BF, tag='w16')
                nc.vector.tensor_copy(w16[:], w[:])
                wpay = pool.tile([128, nch, 128], BF, tag='wpay')
                wv = w16[:]
                bc = bass.AP(tensor=wv.tensor, offset=wv.offset,
                             ap=[wv.ap[0], wv.ap[1], [0, 128]])
                nc.vector.tensor_tensor(out=wpay[:], in0=pay[:], in1=bc,
                                        op=OP.mult)
                aggp = pps.tile([128, 512], FT, tag='aggp')
                zp = pps.tile([1, 512], FT, tag='zp')
                for c in range(nch):
                    oh = pool.tile([128, 512], BF, tag='oh', bufs=2)
                    nc.vector.tensor_scalar(
                        out=oh[:, :ndst], in0=iotaf[:, :ndst],
                        scalar1=dstf[:, dst_off + c:dst_off + c + 1],
                        scalar2=None, op0=OP.is_equal)
                    nc.tensor.matmul(aggp[:, :ndst], lhsT=wpay[:, c, :],
                                     rhs=oh[:, :ndst], start=(c == 0),
                                     stop=(c == nch - 1))
                    nc.tensor.matmul(zp[:, :ndst], lhsT=w16[:, c:c + 1],
                                     rhs=oh[:, :ndst], start=(c == 0),
                                     stop=(c == nch - 1))
                zsb = pool.tile([1, 512], FT, tag='zsb')
                nc.vector.tensor_copy(zsb[:, :ndst], zp[:, :ndst])
                zr = pool.tile([1, 512], FT, tag='zr')
                nc.vector.reciprocal(zr[:, :ndst], zsb[:, :ndst])
                rbp = pps.tile([128, 512], FT, tag='rbp')
                nc.tensor.matmul(rbp[:, :ndst], lhsT=ones1[:],
                                 rhs=zr[:, :ndst], start=True, stop=True)
                rb = pool.tile([128, 512], FT, tag='rb')
                nc.vector.tensor_copy(rb[:, :ndst], rbp[:, :ndst])
                aggn = pool.tile([128, 512], BF, tag='aggn')
                nc.vector.tensor_mul(aggn[:, :ndst], aggp[:, :ndst],
                                     rb[:, :ndst])
                rp = pps.tile([128, 512], FT, tag='rp')
                nc.tensor.matmul(rp[:, :ndst], lhsT=gw16[:],
                                 rhs=aggn[:, :ndst], start=True, stop=True)
                rl = pool.tile([128, 512], BF, tag='rl')
                nc.scalar.activation(out=rl[:, :ndst], in_=rp[:, :ndst],
                                     func=AF.Relu, bias=gb_sb[:])
                rn = _rup(res_n, 128)
                fdr = pool.tile([128, rn], BF, tag='fdr')
                gather_t(fdr[:].rearrange('p (o n) -> p o n', o=1), 0, tab,
                         res_seg, rn)
                outT = pool.tile([128, rn], BF, tag='outT')
                if rn > ndst:
                    nc.vector.memset(outT[:, ndst:], 0.0)
                nc.vector.tensor_add(outT[:, :ndst], fdr[:, :ndst],
                                     rl[:, :ndst])
                return outT

            if PHASE < 4:
                raise _PhaseDone()
            with (
                tc.tile_pool(name='gat', bufs=1) as gp0,
                tc.tile_pool(name='gat_ps', bufs=1, space='PSUM') as gps,
            ):
                f1T = gat_layer(gp0, gps, FTAB, 'g0pay', 'g0fd', NCH0, 0,
                                DST0_PER_CORE, gw0_16, gb0_sb, 'g0res', B1)
                a2 = gp0.tile([128, 3, 128], BF)
                for j in range(3):
                    pt = gps.tile([128, 128], BF, tag='tp2', bufs=2)
                    nc.tensor.transpose(pt[:], f1T[:, j * 128:(j + 1) * 128],
                                        ident[:])
                    nc.vector.tensor_copy(a2[:, j, :], pt[:])
                nc.sync.dma_start(cont_ap(AG2IN, 3), a2[:])
                nc.gpsimd.collective_compute(
                    'AllGather', OP.bypass,
                    replica_groups=[list(range(NCORES))],
                    ins=[AG2IN.opt()], outs=[F1TAB.opt()])

                f2T = gat_layer(gp0, gps, F1TAB, 'g1pay', 'g1fd', NCH1, NCH0,
                                N2, gw1_16, gb1_sb, 'g1res', 512)

                curT = gp0.tile([128, 512], BF)
                gather_t(curT[:].rearrange('p (o n) -> p o n', o=1), 0, FTAB,
                         'cur', 512)
                srp = gps.tile([128, 512], FT, tag='srp')
                nc.tensor.matmul(srp[:], lhsT=w2_16[:, 0, :], rhs=curT[:],
                                 start=True, stop=False)
                nc.tensor.matmul(srp[:], lhsT=w2_16[:, 1, :], rhs=f2T[:, :512],
                                 start=False, stop=True)
                sr16 = glob.tile([128, 512], BF)
                nc.vector.tensor_copy(sr16[:], srp[:])

            # ================= logits =================
            if PHASE < 5:
                raise _PhaseDone()
            with (
                tc.tile_pool(name='lg', bufs=1) as lp,
                tc.tile_pool(name='lg_o', bufs=4) as lop,
                tc.tile_pool(name='lg_ps', bufs=4, space='PSUM') as lps,
            ):
                ls16 = renorm(lp, ishard, JL, rows=LSH)
                nc.sync.dma_start(cont_ap(LSHARD, JL), ls16[:])
                itemT = lp.tile([128, LSHP], BF)
                gather_t(itemT[:].rearrange('p (o n) -> p o n', o=1), 0,
                         LSHARD, 'lsh', LSHP)
                for m in range(4):
                    for n in range((LSH + 511) // 512):
                        cs = n * 512
                        cw = min(512, LSH - cs)
                        ps = lps.tile([128, 512], FT, tag='lgps')
                        nc.tensor.matmul(ps[:, :cw],
                                         lhsT=sr16[:, m * 128:(m + 1) * 128],
                                         rhs=itemT[:, cs:cs + cw],
                                         start=True, stop=True)
                        ob = lop.tile([128, 512], FT, tag='ob')
                        nc.vector.tensor_copy(ob[:, :cw], ps[:, :cw])
                        nc.sync.dma_start(
                            bass.AP(tensor=out_p, offset=m * 128 * LSH + cs,
                                    ap=[[LSH, 128], [1, cw]]),
                            ob[:, :cw])

            ctx.__exit__(None, None, None)
        except _PhaseDone:
            ctx.__exit__(None, None, None)
    nc.compile()
    return nc


_CACHE = {}


def prepare(inputs):
    in_maps, meta = host_prep(inputs)
    import os
    key = (meta['NL'], meta['VC'], meta['E0C'], tuple(meta['act']),
           os.environ.get('KPHASE', '9'), os.environ.get('KSUB', '9'),
           os.environ.get('KT', '99'), os.environ.get('KCH', '9'))
    if key not in _CACHE:
        _CACHE[key] = build_program(meta)
    return _CACHE[key], in_maps, meta


def kernel(**inputs):
    from concourse.bass_utils import run_bass_kernel_spmd
    nc, in_maps, meta = prepare(inputs)
    res = run_bass_kernel_spmd(nc, in_maps, list(range(NCORES)))
    out = np.concatenate([res.results[k]['out'] for k in range(NCORES)],
                         axis=1)
    return np.ascontiguousarray(out.astype(np.float32))



# revision 10
# speedup vs baseline: 1.0314x; 1.0314x over previous
"""DGRec kernel for 8 TRN2 NeuronCores (Bass/Tile).

Strategy (v2):
  - Host: index-only prep + table row-selection sharding. Live-session
    pruning, sessions sorted by lens desc and dealt round-robin so every
    core shares one static shrinking-prefix LSTM schedule. LSTM token
    embeddings are host-expanded per (step, session) into a contiguous
    bf16 buffer (no device gathers for x); renorm happens on device per
    step, fused with a scale-folded PE transpose (matmul vs diag(rcp)).
  - Device: LSTM gates via 4-slot PSUM matmuls, vector bias add + batched
    sigmoid/tanh ACT (gates reordered i,f,o,g so the 3 sigmoids batch),
    bf16 DVE state updates. GAT0 edge-sharded by dst with one-hot
    PE expansion of fd rows (no per-edge fd gather). GAT1 dst-sharded
    8-ways + tiny f2 AllGather. Logits item table renormed + PE-transposed
    fully in SBUF (no DRAM roundtrip/gather). fp16 output, host upcast.
"""
import sys
sys.path.insert(0, '/opt/trn_rl_repo')

import numpy as np
import ml_dtypes

BF16 = ml_dtypes.bfloat16


class _PhaseDone(Exception):
    pass


NCORES = 8
D = 128
T = 20
N0 = 25600
N1 = 2560
N2 = 512
E1 = 5120
NI = 50000
CURB = 128          # cur block rows per core (slot CURB-1 reserved zero row)
DST0_PER_CORE = N1 // NCORES  # 320
DST1_PER_CORE = N2 // NCORES  # 64
B1 = 384            # feat1 block rows per core (320 padded to 384)
ZR1 = 383           # zero row in F1TAB (core 0 block, col 383 of f1T)
CH = 384            # LSTM gate chunk columns
GPERM = [0, 1, 3, 2]  # gate order i,f,o,g (from reference i,f,g,o)


def _rup(x, m):
    return (int(x) + m - 1) // m * m


def _wrap16(idx):
    """[n] int -> [128, n//16] int16: idx i at [i%16, i//16], replicated x8."""
    idx = np.asarray(idx, np.int16)
    n = len(idx)
    assert n % 16 == 0
    a = idx.reshape(n // 16, 16).T  # [16, n//16]
    return np.tile(a, (8, 1))


def _wrap_rows(rows, J):
    """[n<=128J, D] -> [128J, D] cont-ap layout: row (s%128)*J + s//128 = rows[s]."""
    n = len(rows)
    out = np.zeros((128 * J, D), rows.dtype)
    s = np.arange(n)
    out[(s % 128) * J + s // 128] = rows
    return out


def _perm_gates(w):
    """Reorder 4*D gate rows from (i,f,g,o) to (i,f,o,g)."""
    c = [w[g * D:(g + 1) * D] for g in range(4)]
    return np.concatenate([c[g] for g in GPERM], axis=0)


def host_prep(inputs):
    lens = np.asarray(inputs['lens']).astype(np.int64)
    seqs = np.asarray(inputs['padded_seqs']).astype(np.int64)
    uids = np.asarray(inputs['uids']).astype(np.int64)
    cur_sidx = np.asarray(inputs['cur_sidx']).astype(np.int64)
    src0 = np.asarray(inputs['src0']).astype(np.int64)
    dst0 = np.asarray(inputs['dst0']).astype(np.int64)
    idx0 = np.asarray(inputs['idx0']).astype(np.int64)
    src1 = np.asarray(inputs['src1']).astype(np.int64)
    dst1 = np.asarray(inputs['dst1']).astype(np.int64)
    idx1 = np.asarray(inputs['idx1']).astype(np.int64)
    user_emb = np.asarray(inputs['user_emb'], np.float32)
    item_emb = np.ascontiguousarray(
        np.asarray(inputs['item_emb'], np.float32)).astype(BF16)
    item_emb[0] = 0.0  # padding_idx

    # ---- live sessions, sorted by len desc, per-len-group padded to mult 8
    live_mask = np.zeros(N0, bool)
    live_mask[src0] = True
    live_mask[idx0] = True
    live_mask[cur_sidx] = True
    live = np.where(live_mask)[0]
    order = live[np.argsort(-lens[live], kind='stable')]
    lens_live = lens[order]
    parts = []
    grp_ceil = {}
    for L in range(T, 0, -1):
        grp = order[lens_live == L]
        pad = (-len(grp)) % NCORES
        parts.append(grp)
        grp_ceil[L] = (len(grp) + pad) // NCORES
        if pad:
            parts.append(np.full(pad, -1, np.int64))
    order_p = np.concatenate(parts)
    percore = len(order_p) // NCORES
    NL = _rup(percore, 128)
    extra = NL * NCORES - len(order_p)
    order_p = np.concatenate([order_p, np.full(extra, -1, np.int64)])
    core_sessions = [order_p[k::NCORES] for k in range(NCORES)]
    act = [sum(grp_ceil[L] for L in range(t + 1, T + 1)) for t in range(T)]
    act = [min(a, NL) for a in act]

    # ---- FTAB layout: per-core block [NL feat rows][CURB cur rows]
    BLK = NL + CURB
    ZROW = NL + CURB - 1  # core 0 block, slot CURB-1: reserved all-zero row
    sess2pos = np.full(N0, -1, np.int64)
    core_local = []       # per core: {session: local index}
    JU_ = NL // 128
    for k in range(NCORES):
        sess = core_sessions[k]
        real = sess >= 0
        li = np.where(real)[0]
        sess2pos[sess[real]] = k * BLK + (li % 128) * JU_ + li // 128
        core_local.append({int(s): i for i, s in enumerate(sess) if s >= 0})
    # cur block: per core, unique local cur sessions -> slots 0..cnt-1
    cur_pos = {}
    cur_slot_local = [[] for _ in range(NCORES)]
    for s in np.unique(cur_sidx):
        owner = -1
        for k in range(NCORES):
            if int(s) in core_local[k]:
                owner = k
                break
        assert owner >= 0
        slot = len(cur_slot_local[owner])
        assert slot < 112
        cur_pos[int(s)] = owner * BLK + NL + slot
        cur_slot_local[owner].append(core_local[owner][int(s)])
    # remap: sessions in cur_sidx read hn (feat.at[cur].set(cur))
    for s in np.unique(cur_sidx):
        sess2pos[s] = cur_pos[int(s)]

    # ---- per-step x expansion layout
    a128s = [_rup(max(act[t], 1), 128) for t in range(T)]
    xoff = np.cumsum([0] + a128s).tolist()
    SUMA = xoff[-1]

    # ---- GAT0: edges sharded by dst range, sorted by dst
    g0 = []
    for k in range(NCORES):
        lo, hi = k * DST0_PER_CORE, (k + 1) * DST0_PER_CORE
        e = np.where((dst0 >= lo) & (dst0 < hi))[0]
        e = e[np.argsort(dst0[e], kind='stable')]
        g0.append(e)
    E0C = _rup(max(len(e) for e in g0), 128)
    NCH0 = E0C // 128

    # ---- GAT1: edges sharded by dst range (64 per core), sorted by dst
    g1 = []
    for k in range(NCORES):
        lo, hi = k * DST1_PER_CORE, (k + 1) * DST1_PER_CORE
        e = np.where((dst1 >= lo) & (dst1 < hi))[0]
        e = e[np.argsort(dst1[e], kind='stable')]
        g1.append(e)
    E1C = _rup(max(len(e) for e in g1), 128)
    NCH1 = E1C // 128

    def f1pos(node):
        node = np.asarray(node)
        loc = node % DST0_PER_CORE
        return (node // DST0_PER_CORE) * B1 + (loc % 128) * 3 + loc // 128

    cur_idx = np.array([cur_pos[int(s)] for s in cur_sidx], np.int64)

    LSH = NI // NCORES          # 6250
    LSHP = _rup(LSH, 128)       # 6272
    JL = LSHP // 128

    meta = dict(NL=NL, BLK=BLK, E0C=E0C, NCH0=NCH0, E1C=E1C, NCH1=NCH1,
                LSH=LSH, LSHP=LSHP, act=act, ZROW=ZROW, SUMA=SUMA, xoff=xoff)

    # ---- IDX16 buffer layout (columns of 16-wrapped idx); same offsets all cores
    seg_off = {}
    _w = [0]

    def add_seg(name, n):
        seg_off[name] = _w[0]
        _w[0] += n // 16

    add_seg('g0pay', E0C)
    add_seg('g0res', B1)
    add_seg('g1pay', E1C)
    add_seg('g1fd', 128)
    add_seg('cur', 512)
    add_seg('hnrow', 112)
    W16 = _w[0]
    meta['seg_off'] = seg_off
    meta['W16'] = W16

    in_maps = []
    for k in range(NCORES):
        sess = core_sessions[k]
        smax = np.maximum(sess, 0)

        # x expansion: step t rows wrapped into cont-ap layout
        xseq = np.zeros((SUMA, D), BF16)
        for t in range(T):
            a = act[t]
            if a == 0:
                break
            toks = np.where(sess[:a] >= 0, seqs[smax[:a], t], 0)
            rows = item_emb[toks]
            xseq[xoff[t]:xoff[t + 1]] = _wrap_rows(rows, a128s[t] // 128)

        usr_rows = user_emb[uids[smax]].astype(BF16)
        usr_rows[sess < 0] = 0.0
        usr = _wrap_rows(usr_rows[:NL], NL // 128)

        ishard = _wrap_rows(
            np.asarray(item_emb[1 + k * LSH: 1 + (k + 1) * LSH]), JL)

        idx16 = np.zeros((128, W16), np.int16)

        def put(name, vals):
            v = np.asarray(vals, np.int64)
            assert v.min() >= -1 and v.max() < 32767, (name, v.min(), v.max())
            o = seg_off[name]
            w = _wrap16(v.astype(np.int16))
            idx16[:, o:o + w.shape[1]] = w

        e = g0[k]
        pay = np.full(E0C, ZROW, np.int64)
        dstl = np.full(E0C, -1, np.int64)
        pay[:len(e)] = sess2pos[src0[e]]
        dstl[:len(e)] = dst0[e] - k * DST0_PER_CORE
        put('g0pay', pay)
        res0 = np.full(B1, ZROW, np.int64)
        res0[:DST0_PER_CORE] = sess2pos[
            idx0[k * DST0_PER_CORE:(k + 1) * DST0_PER_CORE]]
        put('g0res', res0)

        e1 = g1[k]
        pay1 = np.full(E1C, ZR1, np.int64)
        dstl1 = np.full(E1C, -1, np.int64)
        pay1[:len(e1)] = f1pos(src1[e1])
        dstl1[:len(e1)] = dst1[e1] - k * DST1_PER_CORE
        put('g1pay', pay1)
        fd1 = np.full(128, ZR1, np.int64)
        fd1[:DST1_PER_CORE] = f1pos(
            idx1[k * DST1_PER_CORE:(k + 1) * DST1_PER_CORE])
        put('g1fd', fd1)
        put('cur', cur_idx)
        hnrow = np.zeros(112, np.int64)
        cs = np.asarray(cur_slot_local[k], np.int64)
        assert len(cs) <= 112
        if len(cs):
            hnrow[:len(cs)] = (cs % 128) * (NL // 128) + cs // 128
        put('hnrow', hnrow)

        # dst-local values for one-hot compare: edge (chunk c, partition p)
        dstl_all = np.full((128, NCH0 + NCH1), -1, np.int32)
        dstl_all[:, :NCH0] = dstl.reshape(NCH0, 128).T
        dstl_all[:, NCH0:] = dstl1.reshape(NCH1, 128).T

        in_maps.append({
            'xseq': xseq,
            'usr': usr,
            'ishard': ishard,
            'WihT': np.ascontiguousarray(
                _perm_gates(np.asarray(inputs['Wih'], np.float32)).T
            ).astype(BF16),
            'WhhT': np.ascontiguousarray(
                _perm_gates(np.asarray(inputs['Whh'], np.float32)).T
            ).astype(BF16),
            'bih': _perm_gates(np.asarray(inputs['bih'], np.float32)[:, None])[:, 0],
            'bhh': _perm_gates(np.asarray(inputs['bhh'], np.float32)[:, None])[:, 0],
            'W1T': np.ascontiguousarray(
                np.asarray(inputs['W1'], np.float32).T).astype(BF16),
            'W2T': np.ascontiguousarray(
                np.asarray(inputs['W2'], np.float32).T).astype(BF16),
            'gW0T': np.ascontiguousarray(
                np.asarray(inputs['gW0'], np.float32).T).astype(BF16),
            'gW1T': np.ascontiguousarray(
                np.asarray(inputs['gW1'], np.float32).T).astype(BF16),
            'gb0': np.asarray(inputs['gb0'], np.float32),
            'gb1': np.asarray(inputs['gb1'], np.float32),
            'idx16': idx16,
            'dstl': dstl_all,
        })
    return in_maps, meta


# ============================ device program ============================

def build_program(meta):
    import os
    PHASE = int(os.environ.get('KPHASE', '9'))
    import contextlib
    import concourse.bass as bass
    import concourse.mybir as mybir
    import concourse.tile as tile
    from concourse import bacc
    from concourse.masks import make_identity

    NL = meta['NL']
    BLK = meta['BLK']
    NCH0 = meta['NCH0']
    NCH1 = meta['NCH1']
    LSH = meta['LSH']
    LSHP = meta['LSHP']
    act = meta['act']
    seg = meta['seg_off']
    W16 = meta['W16']
    xoff = meta['xoff']
    JU = NL // 128
    JL = LSHP // 128
    A0 = _rup(act[0], 128)
    J0 = A0 // 128
    FT = mybir.dt.float32
    BF = mybir.dt.bfloat16
    F16 = mybir.dt.float16
    AF = mybir.ActivationFunctionType
    OP = mybir.AluOpType

    nc = bacc.Bacc("TRN2", target_bir_lowering=False, debug=False,
                   num_devices=NCORES)

    def param(name, shape, dt=FT):
        return nc.declare_dram_parameter(name, list(shape), dt, isOutput=False)

    xseq_p = param('xseq', [meta['SUMA'], D], BF)
    usr_p = param('usr', [NL, D], BF)
    ishard_p = param('ishard', [LSHP, D], BF)
    WihT = param('WihT', [D, 512], BF)
    WhhT = param('WhhT', [D, 512], BF)
    bih = param('bih', [512])
    bhh = param('bhh', [512])
    W1T = param('W1T', [256, D], BF)
    W2T = param('W2T', [256, D], BF)
    gW0T = param('gW0T', [D, D], BF)
    gW1T = param('gW1T', [D, D], BF)
    gb0 = param('gb0', [D])
    gb1 = param('gb1', [D])
    idx16_p = param('idx16', [128, W16], mybir.dt.int16)
    dstl_p = param('dstl', [128, NCH0 + NCH1], mybir.dt.int32)
    out_p = nc.declare_dram_parameter('out', [N2, LSH], F16, isOutput=True)

    def rows_ap(handle_ap, j_count, base_elem=0):
        """view rows [128*j_count, D] of a DRAM tensor as [128, j, D], row=128j+p"""
        t = handle_ap if isinstance(handle_ap, bass.AP) else handle_ap[:]
        return bass.AP(tensor=t.tensor, offset=t.offset + base_elem,
                       ap=[[D, 128], [128 * D, j_count], [1, D]])

    def cont_ap(handle_ap, j_count, base_elem=0):
        """contiguous [128, j, D] view: slot (p, j) -> DRAM row p*j_count + j"""
        t = handle_ap if isinstance(handle_ap, bass.AP) else handle_ap[:]
        return bass.AP(tensor=t.tensor, offset=t.offset + base_elem,
                       ap=[[j_count * D, 128], [D, j_count], [1, D]])

    def bcast_free(ap, n0, nb):
        """[128, n0] AP -> [128, n0, nb] with 0-stride innermost broadcast."""
        return bass.AP(tensor=ap.tensor, offset=ap.offset,
                       ap=[ap.ap[0], [ap.ap[1][0], n0], [0, nb]])

    with tile.TileContext(nc) as tc:
        try:
            ctx = contextlib.ExitStack()
            ctx.__enter__()
            glob = ctx.enter_context(tc.tile_pool(name='glob', bufs=1))
            dram = ctx.enter_context(tc.tile_pool(name='dram', bufs=1,
                                                  space='DRAM'))
            tps = ctx.enter_context(
                tc.tile_pool(name='tps', bufs=2, space='PSUM'))

            HNROWS = dram.tile([NL, D], BF)
            AGIN = dram.tile([BLK, D], BF)
            FTAB = dram.tile([NCORES * BLK, D], BF, addr_space='Shared')
            AG2IN = dram.tile([B1, D], BF)
            F1TAB = dram.tile([NCORES * B1, D], BF, addr_space='Shared')
            AG3IN = dram.tile([DST1_PER_CORE, D], BF)
            F2TAB = dram.tile([N2, D], BF, addr_space='Shared')

            # ---- global constants / index tiles
            idx_sb = glob.tile([128, W16], mybir.dt.int16)
            nc.sync.dma_start(idx_sb[:], idx16_p[:])

            def seg_ap(name, n, off=0):
                o = seg[name] + off // 16
                return idx_sb[:, o:o + n // 16]

            GMAX = 512

            def gather_t(out_full, colbase, tab, name, n, queue=0):
                """transpose-mode gather of n idx (mult 128) from segment
                `name` into out_full[:, 0, colbase:colbase+n], split <=GMAX"""
                for o in range(0, n, GMAX):
                    w = min(GMAX, n - o)
                    nc.gpsimd.dma_gather(
                        out_ap=out_full[:, :, colbase + o:colbase + o + w],
                        in_ap=tab[:], idxs_ap=seg_ap(name, w, o),
                        num_idxs=w, num_idxs_reg=w, elem_size=D,
                        transpose=True, queue_num=queue)

            def gather_rows(out_tile, tab, name, n, queue=0):
                """non-transpose gather of n idx into [128, n//128, 128]"""
                for o in range(0, n, GMAX):
                    w = min(GMAX, n - o)
                    nc.gpsimd.dma_gather(
                        out_ap=out_tile[:, o // 128:(o + w) // 128, :],
                        in_ap=tab[:], idxs_ap=seg_ap(name, w, o),
                        num_idxs=w, num_idxs_reg=w, elem_size=D,
                        transpose=False, queue_num=queue)

            ident = glob.tile([128, 128], BF)
            make_identity(nc, ident[:])
            ident4 = glob.tile([128, 4, 128], BF)
            for g in range(4):
                nc.vector.tensor_copy(ident4[:, g, :], ident[:])
            iota_i = glob.tile([128, 512], mybir.dt.int32)
            nc.gpsimd.iota(iota_i[:], pattern=[[1, 512]], base=0,
                           channel_multiplier=0)
            iotaf = glob.tile([128, 512], FT)
            nc.vector.tensor_copy(iotaf[:], iota_i[:])
            dstl_i = glob.tile([128, NCH0 + NCH1], mybir.dt.int32)
            nc.sync.dma_start(dstl_i[:], dstl_p[:])
            dstf = glob.tile([128, NCH0 + NCH1], FT)
            nc.vector.tensor_copy(dstf[:], dstl_i[:])
            ones1 = glob.tile([1, 128], FT)
            nc.vector.memset(ones1[:], 1.0)

            # ---- weights (already bf16 from host)
            wih16 = glob.tile([128, 512], BF)
            nc.sync.dma_start(wih16[:], WihT[:])
            whh16 = glob.tile([128, 512], BF)
            nc.sync.dma_start(whh16[:], WhhT[:])
            w1_16 = glob.tile([128, 2, 128], BF)
            nc.sync.dma_start(w1_16[:], rows_ap(W1T, 2))
            w2_16 = glob.tile([128, 2, 128], BF)
            nc.sync.dma_start(w2_16[:], rows_ap(W2T, 2))
            gw0_16 = glob.tile([128, 128], BF)
            nc.sync.dma_start(gw0_16[:], gW0T[:])
            gw1_16 = glob.tile([128, 128], BF)
            nc.sync.dma_start(gw1_16[:], gW1T[:])
            gb0_sb = glob.tile([128, 1], FT)
            nc.sync.dma_start(gb0_sb[:], bass.AP(tensor=gb0, offset=0,
                                                 ap=[[1, 128], [1, 1]]))
            gb1_sb = glob.tile([128, 1], FT)
            nc.sync.dma_start(gb1_sb[:], bass.AP(tensor=gb1, offset=0,
                                                 ap=[[1, 128], [1, 1]]))
            bi_sb = glob.tile([128, 4], FT)
            nc.sync.dma_start(bi_sb[:], bass.AP(tensor=bih, offset=0,
                                                ap=[[1, 128], [128, 4]]))
            bh_sb = glob.tile([128, 4], FT)
            nc.sync.dma_start(bh_sb[:], bass.AP(tensor=bhh, offset=0,
                                                ap=[[1, 128], [128, 4]]))
            bias = glob.tile([128, 4], FT)
            nc.vector.tensor_add(bias[:], bi_sb[:], bh_sb[:])

            # ---- renorm rows [128, :J, 128] bf16 -> rcp16 [128, JM] bf16
            def renorm_rcp(pool, stg, J, JM, tag=''):
                sq = pool.tile([128, JM, 128], BF, tag='rn_sq' + tag)
                nc.vector.tensor_mul(sq[:, :J, :], stg[:, :J, :],
                                     stg[:, :J, :])
                sumsq = pool.tile([128, JM], FT, tag='rn_ss' + tag)
                nc.vector.tensor_reduce(out=sumsq[:, :J], in_=sq[:, :J, :],
                                        axis=mybir.AxisListType.X, op=OP.add)
                nrm = pool.tile([128, JM], FT, tag='rn_nrm' + tag)
                nc.scalar.activation(out=nrm[:, :J], in_=sumsq[:, :J],
                                     func=AF.Sqrt)
                nc.vector.tensor_scalar_max(nrm[:, :J], nrm[:, :J], 1e-12)
                rcp = pool.tile([128, JM], FT, tag='rn_rcp' + tag)
                nc.vector.reciprocal(rcp[:, :J], nrm[:, :J])
                nc.vector.tensor_scalar_min(rcp[:, :J], rcp[:, :J], 1.0)
                rcp16 = pool.tile([128, JM], BF, tag='rn_r16' + tag)
                nc.vector.tensor_copy(rcp16[:, :J], rcp[:, :J])
                return rcp16

            # ---- scaled transpose: out[:, 128j+p] = stg[p, j, :] * rcp[p, j]
            def scaled_transpose(pool, outT, stg, rcp16, J, tag=''):
                for g0i in range(0, J, 4):
                    ng = min(4, J - g0i)
                    diag = pool.tile([128, 4, 128], BF, tag='diag' + tag)
                    r = rcp16[:]
                    nc.vector.tensor_tensor(
                        out=diag[:, :ng, :], in0=ident4[:, :ng, :],
                        in1=bass.AP(tensor=r.tensor,
                                    offset=r.offset + g0i * r.ap[1][0],
                                    ap=[r.ap[0], [r.ap[1][0], ng], [0, 128]]),
                        op=OP.mult)
                    tp = tps.tile([128, 4, 128], FT, tag='tp')
                    for j in range(ng):
                        nc.tensor.matmul(tp[:, j, :], lhsT=stg[:, g0i + j, :],
                                         rhs=diag[:, j, :], start=True,
                                         stop=True)
                    nc.vector.tensor_copy(
                        outT[:, (g0i) * 128:(g0i + ng) * 128],
                        tp[:, :ng, :])

            # ---- plain transpose rows->cols: outT[:, 128j+p] = rows[p, j, :]
            def transpose_rows(outT, rows_t, J, cols=None):
                for g0i in range(0, J, 4):
                    ng = min(4, J - g0i)
                    tp = tps.tile([128, 4, 128], FT, tag='tp')
                    for j in range(ng):
                        nc.tensor.matmul(tp[:, j, :],
                                         lhsT=rows_t[:, g0i + j, :],
                                         rhs=ident[:], start=True, stop=True)
                    nc.vector.tensor_copy(
                        outT[:, g0i * 128:(g0i + ng) * 128], tp[:, :ng, :])

            # ---- transpose cols->rows: rows[p, j, :] = srcT[:, 128j+p]
            def transpose_cols(rows_t, srcT, J):
                for g0i in range(0, J, 4):
                    ng = min(4, J - g0i)
                    tp = tps.tile([128, 4, 128], FT, tag='tp')
                    for j in range(ng):
                        nc.tensor.matmul(
                            tp[:, j, :],
                            lhsT=srcT[:, (g0i + j) * 128:(g0i + j + 1) * 128],
                            rhs=ident[:], start=True, stop=True)
                    nc.vector.tensor_copy(rows_t[:, g0i:g0i + ng, :],
                                          tp[:, :ng, :])

            # ================= user renorm -> longT =================
            hT = glob.tile([128, NL], BF)
            cT = glob.tile([128, NL], BF)
            longT = glob.tile([128, NL], BF)
            nc.vector.memset(hT[:], 0.0)
            nc.vector.memset(cT[:], 0.0)

            with tc.tile_pool(name='upre', bufs=1) as up:
                ustg = up.tile([128, JU, 128], BF)
                nc.sync.dma_start(ustg[:], cont_ap(usr_p, JU))
                urcp = renorm_rcp(up, ustg, JU, JU, tag='u')
                scaled_transpose(up, longT, ustg, urcp, JU, tag='u')

            # ================= LSTM =================
            if PHASE < 2:
                raise _PhaseDone()
            with (
                tc.tile_pool(name='lstm_x', bufs=3) as xp,
                tc.tile_pool(name='lstm_g', bufs=3) as sp,
                tc.tile_pool(name='lstm_ps', bufs=2, space='PSUM') as gp,
            ):
                xT = glob.tile([128, A0], BF)
                for t in range(T):
                    a = act[t]
                    if a == 0:
                        break
                    a128 = _rup(a, 128)
                    J = a128 // 128
                    stg = xp.tile([128, J0, 128], BF, tag='stg')
                    nc.sync.dma_start(stg[:, :J, :],
                                      cont_ap(xseq_p, J, xoff[t] * D))
                    rcp16 = renorm_rcp(xp, stg, J, J0, tag='x')
                    scaled_transpose(xp, xT, stg, rcp16, J, tag='x')

                    nch = (a + CH - 1) // CH
                    for c in range(nch):
                        cs = c * CH
                        cw = min(CH, a - cs)
                        ce = cs + cw
                        g4 = gp.tile([128, 4, CH], FT, tag='g4')
                        for g in range(4):
                            nc.tensor.matmul(
                                g4[:, g, :cw],
                                lhsT=wih16[:, g * 128:(g + 1) * 128],
                                rhs=xT[:, cs:ce], start=True, stop=(t == 0))
                            if t > 0:
                                nc.tensor.matmul(
                                    g4[:, g, :cw],
                                    lhsT=whh16[:, g * 128:(g + 1) * 128],
                                    rhs=hT[:, cs:ce], start=False, stop=True)
                        # bias add on vector, then batched activations
                        gb = sp.tile([128, 4, CH], BF, tag='gb')
                        nc.vector.tensor_tensor(
                            out=gb[:, :, :cw], in0=g4[:, :, :cw],
                            in1=bcast_free(bias[:], 4, cw), op=OP.add)
                        sg = sp.tile([128, 4, CH], BF, tag='sg')
                        nc.scalar.activation(out=sg[:, :3, :cw],
                                             in_=gb[:, :3, :cw],
                                             func=AF.Sigmoid)
                        nc.scalar.activation(out=sg[:, 3, :cw],
                                             in_=gb[:, 3, :cw], func=AF.Tanh)
                        # state update: i=0 f=1 o=2 g=3
                        if t > 0:
                            tmp = sp.tile([128, CH], BF, tag='tmp')
                            nc.vector.tensor_mul(tmp[:, :cw], sg[:, 0, :cw],
                                                 sg[:, 3, :cw])
                            nc.vector.tensor_mul(cT[:, cs:ce], cT[:, cs:ce],
                                                 sg[:, 1, :cw])
                            nc.vector.tensor_add(cT[:, cs:ce], cT[:, cs:ce],
                                                 tmp[:, :cw])
                        else:
                            nc.vector.tensor_mul(cT[:, cs:ce], sg[:, 0, :cw],
                                                 sg[:, 3, :cw])
                        th = sp.tile([128, CH], BF, tag='th')
                        nc.scalar.activation(out=th[:, :cw], in_=cT[:, cs:ce],
                                             func=AF.Tanh)
                        nc.vector.tensor_mul(hT[:, cs:ce], sg[:, 2, :cw],
                                             th[:, :cw])

            # ============ feat + transposes + AG1 ============
            if PHASE < 3:
                raise _PhaseDone()
            with (
                tc.tile_pool(name='feat', bufs=1) as fp,
                tc.tile_pool(name='feat_ps', bufs=2, space='PSUM') as fps,
            ):
                featT = fp.tile([128, NL], BF)
                for c in range((NL + 511) // 512):
                    cs = c * 512
                    cw = min(512, NL - cs)
                    ps = fps.tile([128, 512], FT, tag='fps')
                    nc.tensor.matmul(ps[:, :cw], lhsT=w1_16[:, 0, :],
                                     rhs=longT[:, cs:cs + cw], start=True,
                                     stop=False)
                    nc.tensor.matmul(ps[:, :cw], lhsT=w1_16[:, 1, :],
                                     rhs=hT[:, cs:cs + cw], start=False,
                                     stop=True)
                    nc.scalar.activation(out=featT[:, cs:cs + cw],
                                         in_=ps[:, :cw], func=AF.Relu)

                fr = fp.tile([128, JU, 128], BF)
                hr = fp.tile([128, JU, 128], BF)
                transpose_cols(fr, featT, JU)
                transpose_cols(hr, hT, JU)
                nc.sync.dma_start(cont_ap(HNROWS, JU), hr[:])
                nc.sync.dma_start(cont_ap(AGIN, JU), fr[:])
                curs = fp.tile([128, 1, 128], BF)
                nc.vector.memset(curs[:], 0.0)
                nc.gpsimd.dma_gather(
                    out_ap=curs[:], in_ap=HNROWS[:],
                    idxs_ap=seg_ap('hnrow', 112),
                    num_idxs=112, num_idxs_reg=112, elem_size=D,
                    transpose=False, queue_num=0)
                ag = AGIN[:]
                nc.sync.dma_start(
                    bass.AP(tensor=ag.tensor, offset=ag.offset + NL * D,
                            ap=[[D, 128], [1, D]]),
                    curs[:, 0, :])
                nc.gpsimd.collective_compute(
                    'AllGather', OP.bypass,
                    replica_groups=[list(range(NCORES))],
                    ins=[AGIN.opt()], outs=[FTAB.opt()])

            # ============ logits table prep (overlaps collective) ============
            if PHASE < 4:
                raise _PhaseDone()
            itemT = glob.tile([128, LSHP], BF)
            with tc.tile_pool(name='lg_i', bufs=1) as lp:
                ls = lp.tile([128, JL, 128], BF)
                nc.sync.dma_start(ls[:], cont_ap(ishard_p, JL))
                lrcp = renorm_rcp(lp, ls, JL, JL, tag='l')
                scaled_transpose(lp, itemT, ls, lrcp, JL, tag='l')

            # ================= GAT layers =================
            def gat_layer(pool, pps, tab, pay_seg, fd_seg, fd_n, nch, dst_off,
                          ndst, gw16, gb_sb):
                """Returns outT [128, rup(ndst,128)] = fd + relu(agg @ W + b)."""
                E = nch * 128
                ndr = _rup(ndst, 128)
                JD = ndr // 128
                pay = pool.tile([128, nch, 128], BF, tag='pay')
                gather_rows(pay, tab, pay_seg, E)
                fdrows = pool.tile([128, _rup(fd_n, 128) // 128, 128], BF,
                                   tag='fdrows')
                gather_rows(fdrows, tab, fd_seg, _rup(fd_n, 128))
                # one-hot expansion of fd rows to edges + score
                fde = pool.tile([128, nch, 128], BF, tag='fde')
                for c in range(nch):
                    oh = pool.tile([128, 512], BF, tag='oh', bufs=2)
                    nc.vector.tensor_scalar(
                        out=oh[:, :ndr], in0=iotaf[:, :ndr],
                        scalar1=dstf[:, dst_off + c:dst_off + c + 1],
                        scalar2=None, op0=OP.is_equal)
                    ohT = pool.tile([128, 4, 128], BF, tag='ohT', bufs=2)
                    tp = tps.tile([128, 4, 128], FT, tag='tp')
                    for j in range(JD):
                        nc.tensor.matmul(tp[:, j, :],
                                         lhsT=oh[:, j * 128:(j + 1) * 128],
                                         rhs=ident[:], start=True, stop=True)
                    nc.vector.tensor_copy(ohT[:, :JD, :], tp[:, :JD, :])
                    fp_ = pps.tile([128, 128], FT, tag='fde_ps', bufs=2)
                    for j in range(JD):
                        nc.tensor.matmul(fp_[:], lhsT=ohT[:, j, :],
                                         rhs=fdrows[:, j, :], start=(j == 0),
                                         stop=(j == JD - 1))
                    nc.vector.tensor_copy(fde[:, c, :], fp_[:])
                score = pool.tile([128, nch], FT, tag='score')
                prod = pool.tile([128, nch, 128], BF, tag='prod')
                nc.vector.tensor_mul(prod[:], pay[:], fde[:])
                nc.vector.tensor_reduce(out=score[:], in_=prod[:],
                                        axis=mybir.AxisListType.X, op=OP.add)
                w = pool.tile([128, nch], FT, tag='w')
                nc.scalar.activation(out=w[:], in_=score[:], func=AF.Exp)
                w16 = pool.tile([128, nch], BF, tag='w16')
                nc.vector.tensor_copy(w16[:], w[:])
                wpay = pool.tile([128, nch, 128], BF, tag='wpay')
                wv = w16[:]
                nc.vector.tensor_tensor(out=wpay[:], in0=pay[:],
                                        in1=bcast_free(wv, nch, 128),
                                        op=OP.mult)
                aggp = pps.tile([128, 512], FT, tag='aggp')
                zp = pps.tile([1, 512], FT, tag='zp')
                for c in range(nch):
                    oh = pool.tile([128, 512], BF, tag='oh', bufs=2)
                    nc.vector.tensor_scalar(
                        out=oh[:, :ndst], in0=iotaf[:, :ndst],
                        scalar1=dstf[:, dst_off + c:dst_off + c + 1],
                        scalar2=None, op0=OP.is_equal)
                    nc.tensor.matmul(aggp[:, :ndst], lhsT=wpay[:, c, :],
                                     rhs=oh[:, :ndst], start=(c == 0),
                                     stop=(c == nch - 1))
                    nc.tensor.matmul(zp[:, :ndst], lhsT=w16[:, c:c + 1],
                                     rhs=oh[:, :ndst], start=(c == 0),
                                     stop=(c == nch - 1))
                zsb = pool.tile([1, 512], FT, tag='zsb')
                nc.vector.tensor_copy(zsb[:, :ndst], zp[:, :ndst])
                zr = pool.tile([1, 512], FT, tag='zr')
                nc.vector.reciprocal(zr[:, :ndst], zsb[:, :ndst])
                rbp = pps.tile([128, 512], FT, tag='mm1')
                nc.tensor.matmul(rbp[:, :ndst], lhsT=ones1[:],
                                 rhs=zr[:, :ndst], start=True, stop=True)
                rb = pool.tile([128, 512], FT, tag='rb')
                nc.vector.tensor_copy(rb[:, :ndst], rbp[:, :ndst])
                aggn = pool.tile([128, 512], BF, tag='aggn')
                nc.vector.tensor_mul(aggn[:, :ndst], aggp[:, :ndst],
                                     rb[:, :ndst])
                rp = pps.tile([128, 512], FT, tag='mm1')
                nc.tensor.matmul(rp[:, :ndst], lhsT=gw16[:],
                                 rhs=aggn[:, :ndst], start=True, stop=True)
                rl = pool.tile([128, 512], BF, tag='rl')
                nc.scalar.activation(out=rl[:, :ndst], in_=rp[:, :ndst],
                                     func=AF.Relu, bias=gb_sb[:])
                fdT = pool.tile([128, 512], BF, tag='fdT')
                transpose_rows(fdT, fdrows, JD)
                outT = pool.tile([128, 512], BF, tag='outT')
                if ndr > ndst:
                    nc.vector.memset(outT[:, ndst:ndr], 0.0)
                nc.vector.tensor_add(outT[:, :ndst], fdT[:, :ndst],
                                     rl[:, :ndst])
                return outT

            if PHASE < 5:
                raise _PhaseDone()
            with (
                tc.tile_pool(name='gat', bufs=1) as gp0,
                tc.tile_pool(name='gat_ps', bufs=1, space='PSUM') as gps,
            ):
                f1T = gat_layer(gp0, gps, FTAB, 'g0pay', 'g0res', B1, NCH0, 0,
                                DST0_PER_CORE, gw0_16, gb0_sb)
                a2 = gp0.tile([128, 3, 128], BF)
                transpose_cols(a2, f1T, 3)
                nc.sync.dma_start(cont_ap(AG2IN, 3), a2[:])
                nc.gpsimd.collective_compute(
                    'AllGather', OP.bypass,
                    replica_groups=[list(range(NCORES))],
                    ins=[AG2IN.opt()], outs=[F1TAB.opt()])

                f2T = gat_layer(gp0, gps, F1TAB, 'g1pay', 'g1fd', 128, NCH1,
                                NCH0, DST1_PER_CORE, gw1_16, gb1_sb)
                # f2 rows (64 local dst) -> AllGather -> full [512, D]
                f2rows = gp0.tile([128, 1, 128], BF)
                transpose_cols(f2rows, f2T, 1)
                a3 = AG3IN[:]
                nc.sync.dma_start(
                    bass.AP(tensor=a3.tensor, offset=a3.offset,
                            ap=[[D, DST1_PER_CORE], [1, D]]),
                    f2rows[:DST1_PER_CORE, 0, :])
                nc.gpsimd.collective_compute(
                    'AllGather', OP.bypass,
                    replica_groups=[list(range(NCORES))],
                    ins=[AG3IN.opt()], outs=[F2TAB.opt()])
                f2stg = gp0.tile([128, 4, 128], BF)
                nc.sync.dma_start(f2stg[:], rows_ap(F2TAB, 4))
                f2Tg = glob.tile([128, 512], BF)
                transpose_rows(f2Tg, f2stg, 4)

                curT = glob.tile([128, 512], BF)
                gather_t(curT[:].rearrange('p (o n) -> p o n', o=1), 0, FTAB,
                         'cur', 512)

            sr16 = glob.tile([128, 512], BF)
            with tc.tile_pool(name='sr_ps', bufs=1, space='PSUM') as srps:
                srp = srps.tile([128, 512], FT, tag='srp')
                nc.tensor.matmul(srp[:], lhsT=w2_16[:, 0, :], rhs=curT[:],
                                 start=True, stop=False)
                nc.tensor.matmul(srp[:], lhsT=w2_16[:, 1, :], rhs=f2Tg[:],
                                 start=False, stop=True)
                nc.vector.tensor_copy(sr16[:], srp[:])

            # ================= logits =================
            if PHASE < 6:
                raise _PhaseDone()
            with (
                tc.tile_pool(name='lg_o', bufs=4) as lop,
                tc.tile_pool(name='lg_ps', bufs=4, space='PSUM') as lps,
            ):
                for m in range(4):
                    for n in range((LSH + 511) // 512):
                        cs = n * 512
                        cw = min(512, LSH - cs)
                        ps = lps.tile([128, 512], FT, tag='lgps')
                        nc.tensor.matmul(ps[:, :cw],
                                         lhsT=sr16[:, m * 128:(m + 1) * 128],
                                         rhs=itemT[:, cs:cs + cw],
                                         start=True, stop=True)
                        ob = lop.tile([128, 512], F16, tag='ob')
                        nc.vector.tensor_copy(ob[:, :cw], ps[:, :cw])
                        nc.sync.dma_start(
                            bass.AP(tensor=out_p, offset=m * 128 * LSH + cs,
                                    ap=[[LSH, 128], [1, cw]]),
                            ob[:, :cw])

            ctx.__exit__(None, None, None)
        except _PhaseDone:
            ctx.__exit__(None, None, None)
    nc.compile()
    return nc


_CACHE = {}


def prepare(inputs):
    in_maps, meta = host_prep(inputs)
    import os
    key = (meta['NL'], meta['E0C'], meta['E1C'], tuple(meta['act']),
           os.environ.get('KPHASE', '9'))
    if key not in _CACHE:
        _CACHE[key] = build_program(meta)
    return _CACHE[key], in_maps, meta


def kernel(**inputs):
    from concourse.bass_utils import run_bass_kernel_spmd
    nc, in_maps, meta = prepare(inputs)
    res = run_bass_kernel_spmd(nc, in_maps, list(range(NCORES)))
    out = np.concatenate([res.results[k]['out'] for k in range(NCORES)],
                         axis=1)
    return np.ascontiguousarray(out.astype(np.float32))


# revision 45
# speedup vs baseline: 114.0476x; 110.5706x over previous
"""DGRec kernel for 8 TRN2 NeuronCores (Bass/Tile).

Strategy (v2):
  - Host: index-only prep + table row-selection sharding. Live-session
    pruning, sessions sorted by lens desc and dealt round-robin so every
    core shares one static shrinking-prefix LSTM schedule. LSTM token
    embeddings are host-expanded per (step, session) into a contiguous
    bf16 buffer (no device gathers for x); renorm happens on device per
    step, fused with a scale-folded PE transpose (matmul vs diag(rcp)).
  - Device: LSTM gates via 4-slot PSUM matmuls, vector bias add + batched
    sigmoid/tanh ACT (gates reordered i,f,o,g so the 3 sigmoids batch),
    bf16 DVE state updates. GAT0 edge-sharded by dst with one-hot
    PE expansion of fd rows (no per-edge fd gather). GAT1 dst-sharded
    8-ways + tiny f2 AllGather. Logits item table renormed + PE-transposed
    fully in SBUF (no DRAM roundtrip/gather). fp16 output, host upcast.
"""
import sys
sys.path.insert(0, '/opt/trn_rl_repo')

import numpy as np
import ml_dtypes

BF16 = ml_dtypes.bfloat16


class _PhaseDone(Exception):
    pass


NCORES = 8
D = 128
T = 20
N0 = 25600
N1 = 2560
N2 = 512
E1 = 5120
NI = 50000
CURB = 128          # cur block rows per core (slot CURB-1 reserved zero row)
DST0_PER_CORE = N1 // NCORES  # 320
DST1_PER_CORE = N2 // NCORES  # 64
B1 = 384            # feat1 block rows per core (320 padded to 384)
ZR1 = 383           # zero row in F1TAB (core 0 block, col 383 of f1T)
CH = 384            # LSTM gate chunk columns
GPERM = [0, 1, 3, 2]  # gate order i,f,o,g (from reference i,f,g,o)


def _rup(x, m):
    return (int(x) + m - 1) // m * m


def _wrap16(idx):
    """[n] int -> [128, n//16] int16: idx i at [i%16, i//16], replicated x8."""
    idx = np.asarray(idx, np.int16)
    n = len(idx)
    assert n % 16 == 0
    a = idx.reshape(n // 16, 16).T  # [16, n//16]
    return np.tile(a, (8, 1))


def _wrap_rows(rows, J):
    """[n<=128J, D] -> [128J, D] cont-ap layout: row (s%128)*J + s//128 = rows[s]."""
    n = len(rows)
    out = np.zeros((128 * J, D), rows.dtype)
    s = np.arange(n)
    out[(s % 128) * J + s // 128] = rows
    return out


def _perm_gates(w):
    """Reorder 4*D gate rows from (i,f,g,o) to (i,f,o,g)."""
    c = [w[g * D:(g + 1) * D] for g in range(4)]
    return np.concatenate([c[g] for g in GPERM], axis=0)


def host_prep(inputs):
    lens = np.asarray(inputs['lens']).astype(np.int64)
    seqs = np.asarray(inputs['padded_seqs']).astype(np.int64)
    uids = np.asarray(inputs['uids']).astype(np.int64)
    cur_sidx = np.asarray(inputs['cur_sidx']).astype(np.int64)
    src0 = np.asarray(inputs['src0']).astype(np.int64)
    dst0 = np.asarray(inputs['dst0']).astype(np.int64)
    idx0 = np.asarray(inputs['idx0']).astype(np.int64)
    src1 = np.asarray(inputs['src1']).astype(np.int64)
    dst1 = np.asarray(inputs['dst1']).astype(np.int64)
    idx1 = np.asarray(inputs['idx1']).astype(np.int64)
    user_emb = np.asarray(inputs['user_emb'], np.float32)
    item_emb = np.ascontiguousarray(
        np.asarray(inputs['item_emb'], np.float32)).astype(BF16)
    item_emb[0] = 0.0  # padding_idx

    # ---- live sessions, sorted by len desc, per-len-group padded to mult 8
    live_mask = np.zeros(N0, bool)
    live_mask[src0] = True
    live_mask[idx0] = True
    live_mask[cur_sidx] = True
    live = np.where(live_mask)[0]
    order = live[np.argsort(-lens[live], kind='stable')]
    lens_live = lens[order]
    parts = []
    grp_ceil = {}
    for L in range(T, 0, -1):
        grp = order[lens_live == L]
        pad = (-len(grp)) % NCORES
        parts.append(grp)
        grp_ceil[L] = (len(grp) + pad) // NCORES
        if pad:
            parts.append(np.full(pad, -1, np.int64))
    order_p = np.concatenate(parts)
    percore = len(order_p) // NCORES
    NL = _rup(percore, 128)
    extra = NL * NCORES - len(order_p)
    order_p = np.concatenate([order_p, np.full(extra, -1, np.int64)])
    core_sessions = [order_p[k::NCORES] for k in range(NCORES)]
    act = [sum(grp_ceil[L] for L in range(t + 1, T + 1)) for t in range(T)]
    act = [min(a, NL) for a in act]

    # ---- FTAB layout: per-core block [NL feat rows][CURB cur rows]
    BLK = NL + CURB
    ZROW = NL + CURB - 1  # core 0 block, slot CURB-1: reserved all-zero row
    sess2pos = np.full(N0, -1, np.int64)
    core_local = []       # per core: {session: local index}
    JU_ = NL // 128
    for k in range(NCORES):
        sess = core_sessions[k]
        real = sess >= 0
        li = np.where(real)[0]
        sess2pos[sess[real]] = k * BLK + (li % 128) * JU_ + li // 128
        core_local.append({int(s): i for i, s in enumerate(sess) if s >= 0})
    # cur block: per core, unique local cur sessions -> slots 0..cnt-1
    cur_pos = {}
    cur_slot_local = [[] for _ in range(NCORES)]
    for s in np.unique(cur_sidx):
        owner = -1
        for k in range(NCORES):
            if int(s) in core_local[k]:
                owner = k
                break
        assert owner >= 0
        slot = len(cur_slot_local[owner])
        assert slot < 112
        cur_pos[int(s)] = owner * BLK + NL + slot
        cur_slot_local[owner].append(core_local[owner][int(s)])
    # remap: sessions in cur_sidx read hn (feat.at[cur].set(cur))
    for s in np.unique(cur_sidx):
        sess2pos[s] = cur_pos[int(s)]

    # ---- per-step x expansion layout
    a128s = [_rup(max(act[t], 1), 128) for t in range(T)]
    xoff = np.cumsum([0] + a128s).tolist()
    SUMA = xoff[-1]

    # ---- GAT0: edges sharded by dst range, sorted by dst
    g0 = []
    for k in range(NCORES):
        lo, hi = k * DST0_PER_CORE, (k + 1) * DST0_PER_CORE
        e = np.where((dst0 >= lo) & (dst0 < hi))[0]
        e = e[np.argsort(dst0[e], kind='stable')]
        g0.append(e)
    E0C = _rup(max(len(e) for e in g0), 128)
    NCH0 = E0C // 128

    # ---- GAT1: edges sharded by dst range (64 per core), sorted by dst
    g1 = []
    for k in range(NCORES):
        lo, hi = k * DST1_PER_CORE, (k + 1) * DST1_PER_CORE
        e = np.where((dst1 >= lo) & (dst1 < hi))[0]
        e = e[np.argsort(dst1[e], kind='stable')]
        g1.append(e)
    E1C = _rup(max(len(e) for e in g1), 128)
    NCH1 = E1C // 128

    def f1pos(node):
        node = np.asarray(node)
        loc = node % DST0_PER_CORE
        return (node // DST0_PER_CORE) * B1 + (loc % 128) * 3 + loc // 128

    cur_idx = np.array([cur_pos[int(s)] for s in cur_sidx], np.int64)

    LSH = NI // NCORES          # 6250
    LSHP = _rup(LSH, 128)       # 6272
    JL = LSHP // 128

    meta = dict(NL=NL, BLK=BLK, E0C=E0C, NCH0=NCH0, E1C=E1C, NCH1=NCH1,
                LSH=LSH, LSHP=LSHP, act=act, ZROW=ZROW, SUMA=SUMA, xoff=xoff)

    # ---- IDX16 buffer layout (columns of 16-wrapped idx); same offsets all cores
    seg_off = {}
    _w = [0]

    def add_seg(name, n):
        seg_off[name] = _w[0]
        _w[0] += n // 16

    add_seg('g0pay', E0C)
    add_seg('g0fd', E0C)
    add_seg('g0res', B1)
    add_seg('g1pay', E1C)
    add_seg('g1fde', E1C)
    add_seg('g1fd', 128)
    add_seg('cur', 512)
    add_seg('hnrow', 112)
    W16 = _w[0]
    meta['seg_off'] = seg_off
    meta['W16'] = W16

    in_maps = []
    for k in range(NCORES):
        sess = core_sessions[k]
        smax = np.maximum(sess, 0)

        # x expansion: transposed [D, SUMA]; col xoff[t]+s = step t, session s
        xseq = np.zeros((D, SUMA), BF16)
        for t in range(T):
            a = act[t]
            if a == 0:
                break
            toks = np.where(sess[:a] >= 0, seqs[smax[:a], t], 0)
            xseq[:, xoff[t]:xoff[t] + a] = item_emb[toks].T

        usr_rows = user_emb[uids[smax]].astype(BF16)
        usr_rows[sess < 0] = 0.0
        usr = _wrap_rows(usr_rows[:NL], NL // 128)

        ishard = _wrap_rows(
            np.asarray(item_emb[1 + k * LSH: 1 + (k + 1) * LSH]), JL)

        idx16 = np.zeros((128, W16), np.int16)

        def put(name, vals):
            v = np.asarray(vals, np.int64)
            assert v.min() >= -1 and v.max() < 32767, (name, v.min(), v.max())
            o = seg_off[name]
            w = _wrap16(v.astype(np.int16))
            idx16[:, o:o + w.shape[1]] = w

        e = g0[k]
        pay = np.full(E0C, ZROW, np.int64)
        fde = np.full(E0C, ZROW, np.int64)
        dstl = np.full(E0C, -1, np.int64)
        pay[:len(e)] = sess2pos[src0[e]]
        fde[:len(e)] = sess2pos[idx0[dst0[e]]]
        dstl[:len(e)] = dst0[e] - k * DST0_PER_CORE
        put('g0pay', pay)
        put('g0fd', fde)
        res0 = np.full(B1, ZROW, np.int64)
        res0[:DST0_PER_CORE] = sess2pos[
            idx0[k * DST0_PER_CORE:(k + 1) * DST0_PER_CORE]]
        put('g0res', res0)

        e1 = g1[k]
        pay1 = np.full(E1C, ZR1, np.int64)
        fde1 = np.full(E1C, ZR1, np.int64)
        dstl1 = np.full(E1C, -1, np.int64)
        pay1[:len(e1)] = f1pos(src1[e1])
        fde1[:len(e1)] = f1pos(idx1[dst1[e1]])
        dstl1[:len(e1)] = dst1[e1] - k * DST1_PER_CORE
        put('g1pay', pay1)
        put('g1fde', fde1)
        fd1 = np.full(128, ZR1, np.int64)
        fd1[:DST1_PER_CORE] = f1pos(
            idx1[k * DST1_PER_CORE:(k + 1) * DST1_PER_CORE])
        put('g1fd', fd1)
        put('cur', cur_idx)
        hnrow = np.zeros(112, np.int64)
        cs = np.asarray(cur_slot_local[k], np.int64)
        assert len(cs) <= 112
        if len(cs):
            hnrow[:len(cs)] = (cs % 128) * (NL // 128) + cs // 128
        put('hnrow', hnrow)

        # dst-local values for one-hot compare: edge (chunk c, partition p)
        dstl_all = np.full((128, NCH0 + NCH1), -1, np.int32)
        dstl_all[:, :NCH0] = dstl.reshape(NCH0, 128).T
        dstl_all[:, NCH0:] = dstl1.reshape(NCH1, 128).T

        in_maps.append({
            'xseq': xseq,
            'usr': usr,
            'ishard': ishard,
            'WihT': np.ascontiguousarray(
                _perm_gates(np.asarray(inputs['Wih'], np.float32)).T
            ).astype(BF16),
            'WhhT': np.ascontiguousarray(
                _perm_gates(np.asarray(inputs['Whh'], np.float32)).T
            ).astype(BF16),
            'bih': _perm_gates(np.asarray(inputs['bih'], np.float32)[:, None])[:, 0],
            'bhh': _perm_gates(np.asarray(inputs['bhh'], np.float32)[:, None])[:, 0],
            'W1T': np.ascontiguousarray(
                np.asarray(inputs['W1'], np.float32).T).astype(BF16),
            'W2T': np.ascontiguousarray(
                np.asarray(inputs['W2'], np.float32).T).astype(BF16),
            'gW0T': np.ascontiguousarray(
                np.asarray(inputs['gW0'], np.float32).T).astype(BF16),
            'gW1T': np.ascontiguousarray(
                np.asarray(inputs['gW1'], np.float32).T).astype(BF16),
            'gb0': np.asarray(inputs['gb0'], np.float32),
            'gb1': np.asarray(inputs['gb1'], np.float32),
            'idx16': idx16,
            'dstl': dstl_all,
        })
    return in_maps, meta


# ============================ device program ============================

def build_program(meta):
    import os
    PHASE = int(os.environ.get('KPHASE', '9'))
    import contextlib
    import concourse.bass as bass
    import concourse.mybir as mybir
    import concourse.tile as tile
    from concourse import bacc
    from concourse.masks import make_identity

    NL = meta['NL']
    BLK = meta['BLK']
    NCH0 = meta['NCH0']
    NCH1 = meta['NCH1']
    LSH = meta['LSH']
    LSHP = meta['LSHP']
    act = meta['act']
    seg = meta['seg_off']
    W16 = meta['W16']
    xoff = meta['xoff']
    JU = NL // 128
    JL = LSHP // 128
    A0 = _rup(act[0], 128)
    J0 = A0 // 128
    FT = mybir.dt.float32
    BF = mybir.dt.bfloat16
    F16 = mybir.dt.float16
    AF = mybir.ActivationFunctionType
    OP = mybir.AluOpType

    nc = bacc.Bacc("TRN2", target_bir_lowering=False, debug=False,
                   num_devices=NCORES, num_swdge_queues=2)

    def param(name, shape, dt=FT):
        return nc.declare_dram_parameter(name, list(shape), dt, isOutput=False)

    xseq_p = param('xseq', [D, meta['SUMA']], BF)
    usr_p = param('usr', [NL, D], BF)
    ishard_p = param('ishard', [LSHP, D], BF)
    WihT = param('WihT', [D, 512], BF)
    WhhT = param('WhhT', [D, 512], BF)
    bih = param('bih', [512])
    bhh = param('bhh', [512])
    W1T = param('W1T', [256, D], BF)
    W2T = param('W2T', [256, D], BF)
    gW0T = param('gW0T', [D, D], BF)
    gW1T = param('gW1T', [D, D], BF)
    gb0 = param('gb0', [D])
    gb1 = param('gb1', [D])
    idx16_p = param('idx16', [128, W16], mybir.dt.int16)
    dstl_p = param('dstl', [128, NCH0 + NCH1], mybir.dt.int32)
    out_p = nc.declare_dram_parameter('out', [N2, LSH], F16, isOutput=True)

    def rows_ap(handle_ap, j_count, base_elem=0):
        """view rows [128*j_count, D] of a DRAM tensor as [128, j, D], row=128j+p"""
        t = handle_ap if isinstance(handle_ap, bass.AP) else handle_ap[:]
        return bass.AP(tensor=t.tensor, offset=t.offset + base_elem,
                       ap=[[D, 128], [128 * D, j_count], [1, D]])

    def cont_ap(handle_ap, j_count, base_elem=0):
        """contiguous [128, j, D] view: slot (p, j) -> DRAM row p*j_count + j"""
        t = handle_ap if isinstance(handle_ap, bass.AP) else handle_ap[:]
        return bass.AP(tensor=t.tensor, offset=t.offset + base_elem,
                       ap=[[j_count * D, 128], [D, j_count], [1, D]])

    def bcast_free(ap, n0, nb):
        """[128, n0] AP -> [128, n0, nb] with 0-stride innermost broadcast."""
        return bass.AP(tensor=ap.tensor, offset=ap.offset,
                       ap=[ap.ap[0], [ap.ap[1][0], n0], [0, nb]])

    with tile.TileContext(nc) as tc:
        try:
            ctx = contextlib.ExitStack()
            ctx.__enter__()
            glob = ctx.enter_context(tc.tile_pool(name='glob', bufs=1))
            dram = ctx.enter_context(tc.tile_pool(name='dram', bufs=1,
                                                  space='DRAM'))
            tps = ctx.enter_context(
                tc.tile_pool(name='tps', bufs=2, space='PSUM'))

            HNROWS = dram.tile([NL, D], BF)
            AGIN = dram.tile([BLK, D], BF)
            FTAB = dram.tile([NCORES * BLK, D], BF, addr_space='Shared')
            AG2IN = dram.tile([B1, D], BF)
            F1TAB = dram.tile([NCORES * B1, D], BF, addr_space='Shared')
            AG3IN = dram.tile([DST1_PER_CORE, D], BF)
            F2TAB = dram.tile([N2, D], BF, addr_space='Shared')

            # ---- global constants / index tiles
            idx_sb = glob.tile([128, W16], mybir.dt.int16)
            nc.sync.dma_start(idx_sb[:], idx16_p[:])

            def seg_ap(name, n, off=0):
                o = seg[name] + off // 16
                return idx_sb[:, o:o + n // 16]

            GMAX = 512

            def gather_t(out_full, colbase, tab, name, n, queue=0):
                """transpose-mode gather of n idx (mult 128) from segment
                `name` into out_full[:, 0, colbase:colbase+n], split <=GMAX"""
                for o in range(0, n, GMAX):
                    w = min(GMAX, n - o)
                    nc.gpsimd.dma_gather(
                        out_ap=out_full[:, :, colbase + o:colbase + o + w],
                        in_ap=tab[:], idxs_ap=seg_ap(name, w, o),
                        num_idxs=w, num_idxs_reg=w, elem_size=D,
                        transpose=True, queue_num=queue)

            def gather_rows(out_tile, tab, name, n, queue=0):
                """non-transpose gather of n idx into [128, n//128, 128]"""
                for o in range(0, n, GMAX):
                    w = min(GMAX, n - o)
                    nc.gpsimd.dma_gather(
                        out_ap=out_tile[:, o // 128:(o + w) // 128, :],
                        in_ap=tab[:], idxs_ap=seg_ap(name, w, o),
                        num_idxs=w, num_idxs_reg=w, elem_size=D,
                        transpose=False, queue_num=queue)

            ident = glob.tile([128, 128], BF)
            make_identity(nc, ident[:])
            ident4 = glob.tile([128, 4, 128], BF)
            for g in range(4):
                nc.vector.tensor_copy(ident4[:, g, :], ident[:])
            ones128 = glob.tile([128, 128], BF)
            nc.vector.memset(ones128[:], 1.0)
            iota_i = glob.tile([128, 512], mybir.dt.int32)
            nc.gpsimd.iota(iota_i[:], pattern=[[1, 512]], base=0,
                           channel_multiplier=0)
            iotaf = glob.tile([128, 512], FT)
            nc.vector.tensor_copy(iotaf[:], iota_i[:])
            dstl_i = glob.tile([128, NCH0 + NCH1], mybir.dt.int32)
            nc.sync.dma_start(dstl_i[:], dstl_p[:])
            dstf = glob.tile([128, NCH0 + NCH1], FT)
            nc.vector.tensor_copy(dstf[:], dstl_i[:])
            ones1 = glob.tile([1, 128], FT)
            nc.vector.memset(ones1[:], 1.0)

            # ---- weights (already bf16 from host)
            wih16 = glob.tile([128, 512], BF)
            nc.sync.dma_start(wih16[:], WihT[:])
            whh16 = glob.tile([128, 512], BF)
            nc.sync.dma_start(whh16[:], WhhT[:])
            w1_16 = glob.tile([128, 2, 128], BF)
            nc.sync.dma_start(w1_16[:], rows_ap(W1T, 2))
            w2_16 = glob.tile([128, 2, 128], BF)
            nc.sync.dma_start(w2_16[:], rows_ap(W2T, 2))
            gw0_16 = glob.tile([128, 128], BF)
            nc.sync.dma_start(gw0_16[:], gW0T[:])
            gw1_16 = glob.tile([128, 128], BF)
            nc.sync.dma_start(gw1_16[:], gW1T[:])
            gb0_sb = glob.tile([128, 1], FT)
            nc.sync.dma_start(gb0_sb[:], bass.AP(tensor=gb0, offset=0,
                                                 ap=[[1, 128], [1, 1]]))
            gb1_sb = glob.tile([128, 1], FT)
            nc.sync.dma_start(gb1_sb[:], bass.AP(tensor=gb1, offset=0,
                                                 ap=[[1, 128], [1, 1]]))
            bi_sb = glob.tile([128, 4], FT)
            nc.sync.dma_start(bi_sb[:], bass.AP(tensor=bih, offset=0,
                                                ap=[[1, 128], [128, 4]]))
            bh_sb = glob.tile([128, 4], FT)
            nc.sync.dma_start(bh_sb[:], bass.AP(tensor=bhh, offset=0,
                                                ap=[[1, 128], [128, 4]]))
            bias = glob.tile([128, 4], FT)
            nc.vector.tensor_add(bias[:], bi_sb[:], bh_sb[:])

            # ---- renorm rows [128, :J, 128] bf16 -> rcp16 [128, JM] bf16
            def renorm_rcp(pool, stg, J, JM, tag=''):
                sq = pool.tile([128, JM, 128], BF, tag='rn_sq' + tag)
                nc.vector.tensor_mul(sq[:, :J, :], stg[:, :J, :],
                                     stg[:, :J, :])
                sumsq = pool.tile([128, JM], FT, tag='rn_ss' + tag)
                nc.vector.tensor_reduce(out=sumsq[:, :J], in_=sq[:, :J, :],
                                        axis=mybir.AxisListType.X, op=OP.add)
                nrm = pool.tile([128, JM], FT, tag='rn_nrm' + tag)
                nc.scalar.activation(out=nrm[:, :J], in_=sumsq[:, :J],
                                     func=AF.Sqrt)
                nc.vector.tensor_scalar_max(nrm[:, :J], nrm[:, :J], 1e-12)
                rcp = pool.tile([128, JM], FT, tag='rn_rcp' + tag)
                nc.vector.reciprocal(rcp[:, :J], nrm[:, :J])
                nc.vector.tensor_scalar_min(rcp[:, :J], rcp[:, :J], 1.0)
                rcp16 = pool.tile([128, JM], BF, tag='rn_r16' + tag)
                nc.vector.tensor_copy(rcp16[:, :J], rcp[:, :J])
                return rcp16

            # ---- scaled transpose: out[:, 128j+p] = stg[p, j, :] * rcp[p, j]
            def scaled_transpose(pool, outT, stg, rcp16, J, tag=''):
                for g0i in range(0, J, 4):
                    ng = min(4, J - g0i)
                    diag = pool.tile([128, 4, 128], BF, tag='diag' + tag)
                    r = rcp16[:]
                    nc.vector.tensor_tensor(
                        out=diag[:, :ng, :], in0=ident4[:, :ng, :],
                        in1=bass.AP(tensor=r.tensor,
                                    offset=r.offset + g0i * r.ap[1][0],
                                    ap=[r.ap[0], [r.ap[1][0], ng], [0, 128]]),
                        op=OP.mult)
                    tp = tps.tile([128, 4, 128], FT, tag='tp')
                    for j in range(ng):
                        nc.tensor.matmul(tp[:, j, :], lhsT=stg[:, g0i + j, :],
                                         rhs=diag[:, j, :], start=True,
                                         stop=True)
                    nc.vector.tensor_copy(
                        outT[:, (g0i) * 128:(g0i + ng) * 128],
                        tp[:, :ng, :])

            # ---- plain transpose rows->cols: outT[:, 128j+p] = rows[p, j, :]
            def transpose_rows(outT, rows_t, J, cols=None):
                for g0i in range(0, J, 4):
                    ng = min(4, J - g0i)
                    tp = tps.tile([128, 4, 128], FT, tag='tp')
                    for j in range(ng):
                        nc.tensor.matmul(tp[:, j, :],
                                         lhsT=rows_t[:, g0i + j, :],
                                         rhs=ident[:], start=True, stop=True)
                    nc.vector.tensor_copy(
                        outT[:, g0i * 128:(g0i + ng) * 128], tp[:, :ng, :])

            # ---- transpose cols->rows: rows[p, j, :] = srcT[:, 128j+p]
            def transpose_cols(rows_t, srcT, J):
                for g0i in range(0, J, 4):
                    ng = min(4, J - g0i)
                    tp = tps.tile([128, 4, 128], FT, tag='tp')
                    for j in range(ng):
                        nc.tensor.matmul(
                            tp[:, j, :],
                            lhsT=srcT[:, (g0i + j) * 128:(g0i + j + 1) * 128],
                            rhs=ident[:], start=True, stop=True)
                    nc.vector.tensor_copy(rows_t[:, g0i:g0i + ng, :],
                                          tp[:, :ng, :])

            # ================= user renorm -> longT =================
            hT = glob.tile([128, NL], BF)
            cT = glob.tile([128, NL], BF)
            longT = glob.tile([128, NL], BF)
            nc.vector.memset(hT[:], 0.0)
            nc.vector.memset(cT[:], 0.0)

            with tc.tile_pool(name='upre', bufs=1) as up:
                ustg = up.tile([128, JU, 128], BF)
                nc.sync.dma_start(ustg[:], cont_ap(usr_p, JU))
                urcp = renorm_rcp(up, ustg, JU, JU, tag='u')
                scaled_transpose(up, longT, ustg, urcp, JU, tag='u')

            # xhat buffer: renorm of xseq done per-step, interleaved with LSTM
            if PHASE < 2:
                raise _PhaseDone()
            SUMA = meta['SUMA']
            xhat = glob.tile([128, SUMA], BF)
            xs = xseq_p[:]
            epsb = glob.tile([128, 1], FT)
            nc.vector.memset(epsb[:], 1e-24)
            bias_row = glob.tile([1, 512], BF)
            bi_r = glob.tile([1, 512], FT)
            bh_r = glob.tile([1, 512], FT)
            nc.sync.dma_start(bi_r[:], bass.AP(tensor=bih, offset=0,
                                               ap=[[1, 1], [1, 512]]))
            nc.sync.dma_start(bh_r[:], bass.AP(tensor=bhh, offset=0,
                                               ap=[[1, 1], [1, 512]]))
            nc.vector.tensor_add(bias_row[:], bi_r[:], bh_r[:])
            ones_row = glob.tile([1, 512], BF)
            nc.vector.memset(ones_row[:], 1.0)

            # ---- upfront renorm of the whole xseq (keeps Sqrt table
            # loads contiguous; gpsimd/vector/scalar split)
            with tc.tile_pool(name='xnorm', bufs=4) as xn:
                for cs in range(0, SUMA, 512):
                    cw = min(512, SUMA - cs)
                    xraw = xn.tile([128, 512], BF, tag='xraw')
                    nc.sync.dma_start(
                        xraw[:, :cw],
                        bass.AP(tensor=xs.tensor, offset=xs.offset + cs,
                                ap=[[SUMA, 128], [1, cw]]))
                    sq = xn.tile([128, 512], BF, tag='xsq')
                    nc.gpsimd.tensor_mul(sq[:, :cw], xraw[:, :cw],
                                         xraw[:, :cw])
                    nps = tps.tile([128, 4, 128], FT, tag='tp')
                    n_ = nps[:]
                    psf = bass.AP(tensor=n_.tensor, offset=n_.offset,
                                  ap=[n_.ap[0], [1, cw]])
                    nc.tensor.matmul(psf, lhsT=ones128[:], rhs=sq[:, :cw],
                                     start=True, stop=True)
                    sr_ = xn.tile([128, 512], FT, tag='sr_')
                    nc.scalar.activation(out=sr_[:, :cw], in_=psf,
                                         func=AF.Sqrt, bias=epsb[:])
                    rs = xn.tile([128, 512], FT, tag='rs')
                    nc.vector.reciprocal_approx_fast(rs[:, :cw], sr_[:, :cw])
                    with nc.allow_low_precision(reason='bf16 renorm'):
                        nc.vector.scalar_tensor_tensor(
                            out=xhat[:, cs:cs + cw],
                            in0=rs[:, :cw], scalar=1.0, in1=xraw[:, :cw],
                            op0=OP.min, op1=OP.mult)

            # ========== SWDGE preps: desc-gen overlaps the LSTM ==========
            # (prepare_only path kept for experiments; broken on this
            # runtime — deferred-completion waits misfire -> NaNs)
            PREP = int(os.environ.get('KPREP', '0'))
            _semn = [0]

            def prep_rows(out_tile, tab, name, n, queue):
                for o in range(0, n, GMAX):
                    w = min(GMAX, n - o)
                    sem = None
                    if PREP:
                        sem = nc.alloc_semaphore(f'swdge{_semn[0]}')
                        _semn[0] += 1
                    nc.gpsimd.dma_gather(
                        out_ap=out_tile[:, o // 128:(o + w) // 128, :],
                        in_ap=tab[:], idxs_ap=seg_ap(name, w, o),
                        num_idxs=w, num_idxs_reg=w, elem_size=D,
                        transpose=False, queue_num=queue,
                        prepare_only=bool(PREP), sem=sem)

            pay0 = glob.tile([128, NCH0, 128], BF)
            fde0 = glob.tile([128, NCH0, 128], BF)
            fdrows0 = glob.tile([128, 3, 128], BF)
            curT = glob.tile([128, 1, 512], BF)
            pay1 = glob.tile([128, NCH1, 128], BF)
            fde1 = glob.tile([128, NCH1, 128], BF)
            fdrows1 = glob.tile([128, 1, 128], BF)

            def gat0_gathers():
                prep_rows(pay0, FTAB, 'g0pay', NCH0 * 128, 0)
                prep_rows(fde0, FTAB, 'g0fd', NCH0 * 128, 0)
                prep_rows(fdrows0, FTAB, 'g0res', B1, 0)

            def cur_gather():
                nc.gpsimd.dma_gather(
                    out_ap=curT[:], in_ap=FTAB[:], idxs_ap=seg_ap('cur', 512),
                    num_idxs=512, num_idxs_reg=512, elem_size=D,
                    transpose=True, queue_num=0)

            def gat1_gathers():
                prep_rows(pay1, F1TAB, 'g1pay', NCH1 * 128, 1)
                prep_rows(fde1, F1TAB, 'g1fde', NCH1 * 128, 1)
                prep_rows(fdrows1, F1TAB, 'g1fd', 128, 1)

            if PREP:
                gat0_gathers()
                gat1_gathers()
            # read-back scratch: orders each trigger after its collective
            # (the collective inst is async; RAW via a 1-row DMA + a gpsimd
            # op in front of the trigger on the same FIFO)
            scr0 = glob.tile([1, D], BF)
            scr1 = glob.tile([1, D], BF)
            scrd = glob.tile([1, D], BF)

            def order_after(tab, scr):
                t_ = tab[:]
                nc.sync.dma_start(
                    scr[:], bass.AP(tensor=t_.tensor, offset=t_.offset,
                                    ap=[[D, 1], [1, D]]))
                nc.gpsimd.tensor_copy(scrd[:], scr[:])

            # ================= LSTM (renorm interleaved) =================
            if PHASE < 3:
                raise _PhaseDone()
            with (
                tc.tile_pool(name='lstm_g', bufs=3) as sp,
                tc.tile_pool(name='lstm_ps', bufs=2, space='PSUM') as gp,
            ):
                for t in range(T):
                    a = act[t]
                    if a == 0:
                        break
                    xo = xoff[t]
                    nch = (a + CH - 1) // CH
                    for c in range(nch):
                        cs = c * CH
                        cw = min(CH, a - cs)
                        ce = cs + cw
                        g4 = gp.tile([128, 4, CH], FT, tag='g4')
                        for g in range(4):
                            nc.tensor.matmul(
                                g4[:, g, :cw],
                                lhsT=bias_row[:, g * 128:(g + 1) * 128],
                                rhs=ones_row[:, :cw], start=True, stop=False)
                            nc.tensor.matmul(
                                g4[:, g, :cw],
                                lhsT=wih16[:, g * 128:(g + 1) * 128],
                                rhs=xhat[:, xo + cs:xo + ce],
                                start=False, stop=(t == 0))
                            if t > 0:
                                nc.tensor.matmul(
                                    g4[:, g, :cw],
                                    lhsT=whh16[:, g * 128:(g + 1) * 128],
                                    rhs=hT[:, cs:ce], start=False, stop=True)
                        sg = sp.tile([128, 4, CH], BF, tag='sg')
                        nc.scalar.activation(out=sg[:, :3, :cw],
                                             in_=g4[:, :3, :cw],
                                             func=AF.Sigmoid)
                        nc.scalar.activation(out=sg[:, 3, :cw],
                                             in_=g4[:, 3, :cw], func=AF.Tanh)
                        # state update: i=0 f=1 o=2 g=3
                        if t > 0:
                            tmp = sp.tile([128, CH], BF, tag='tmp')
                            nc.vector.tensor_mul(tmp[:, :cw], sg[:, 0, :cw],
                                                 sg[:, 3, :cw])
                            nc.vector.tensor_mul(cT[:, cs:ce], cT[:, cs:ce],
                                                 sg[:, 1, :cw])
                            nc.vector.tensor_add(cT[:, cs:ce], cT[:, cs:ce],
                                                 tmp[:, :cw])
                        else:
                            nc.vector.tensor_mul(cT[:, cs:ce], sg[:, 0, :cw],
                                                 sg[:, 3, :cw])
                        th = sp.tile([128, CH], BF, tag='th')
                        nc.scalar.activation(out=th[:, :cw], in_=cT[:, cs:ce],
                                             func=AF.Tanh)
                        nc.vector.tensor_mul(hT[:, cs:ce], sg[:, 2, :cw],
                                             th[:, :cw])

            # ============ feat + transposes + AG1 ============
            if PHASE < 4:
                raise _PhaseDone()
            with (
                tc.tile_pool(name='feat', bufs=1) as fp,
                tc.tile_pool(name='feat_ps', bufs=2, space='PSUM') as fps,
            ):
                featT = fp.tile([128, NL], BF)
                for c in range((NL + 511) // 512):
                    cs = c * 512
                    cw = min(512, NL - cs)
                    ps = fps.tile([128, 512], FT, tag='fps')
                    nc.tensor.matmul(ps[:, :cw], lhsT=w1_16[:, 0, :],
                                     rhs=longT[:, cs:cs + cw], start=True,
                                     stop=False)
                    nc.tensor.matmul(ps[:, :cw], lhsT=w1_16[:, 1, :],
                                     rhs=hT[:, cs:cs + cw], start=False,
                                     stop=True)
                    nc.scalar.activation(out=featT[:, cs:cs + cw],
                                         in_=ps[:, :cw], func=AF.Relu)

                fr = fp.tile([128, JU, 128], BF)
                hr = fp.tile([128, JU, 128], BF)
                transpose_cols(fr, featT, JU)
                transpose_cols(hr, hT, JU)
                nc.sync.dma_start(cont_ap(HNROWS, JU), hr[:])
                nc.sync.dma_start(cont_ap(AGIN, JU), fr[:])
                curs = fp.tile([128, 1, 128], BF)
                nc.vector.memset(curs[:], 0.0)
                nc.gpsimd.dma_gather(
                    out_ap=curs[:], in_ap=HNROWS[:],
                    idxs_ap=seg_ap('hnrow', 112),
                    num_idxs=112, num_idxs_reg=112, elem_size=D,
                    transpose=False, queue_num=0)
                ag = AGIN[:]
                nc.sync.dma_start(
                    bass.AP(tensor=ag.tensor, offset=ag.offset + NL * D,
                            ap=[[D, 128], [1, D]]),
                    curs[:, 0, :])
                nc.gpsimd.collective_compute(
                    'AllGather', OP.bypass,
                    replica_groups=[list(range(NCORES))],
                    ins=[AGIN.opt()], outs=[FTAB.opt()])

            # ============ logits table prep (overlaps collective) ============
            if PHASE < 5:
                raise _PhaseDone()
            itemT = glob.tile([128, LSHP], BF)
            with tc.tile_pool(name='lg_i', bufs=1) as lp:
                ls = lp.tile([128, JL, 128], BF)
                nc.sync.dma_start(ls[:], cont_ap(ishard_p, JL))
                lrcp = renorm_rcp(lp, ls, JL, JL, tag='l')
                scaled_transpose(lp, itemT, ls, lrcp, JL, tag='l')

            # ================= GAT layers =================
            def gat_layer(pool, pps, pay, fde, fdrows, nch, dst_off,
                          ndst, gw16, gb_sb):
                """Returns outT [128, rup(ndst,128)] = fd + relu(agg @ W + b)."""
                ndr = _rup(ndst, 128)
                JD = ndr // 128
                score = pool.tile([128, nch], FT, tag='score')
                prod = pool.tile([128, nch, 128], BF, tag='prod')
                nc.vector.tensor_mul(prod[:], pay[:], fde[:])
                nc.vector.tensor_reduce(out=score[:], in_=prod[:],
                                        axis=mybir.AxisListType.X, op=OP.add)
                w = pool.tile([128, nch], FT, tag='w')
                nc.scalar.activation(out=w[:], in_=score[:], func=AF.Exp)
                w16 = pool.tile([128, nch], BF, tag='w16')
                nc.vector.tensor_copy(w16[:], w[:])
                wpay = pool.tile([128, nch, 128], BF, tag='wpay')
                wv = w16[:]
                nc.vector.tensor_tensor(out=wpay[:], in0=pay[:],
                                        in1=bcast_free(wv, nch, 128),
                                        op=OP.mult)
                aggp = pps.tile([128, 512], FT, tag='aggp')
                zp = pps.tile([1, 512], FT, tag='zp')
                for c in range(nch):
                    oh = pool.tile([128, 512], BF, tag='oh', bufs=2)
                    nc.vector.tensor_scalar(
                        out=oh[:, :ndst], in0=iotaf[:, :ndst],
                        scalar1=dstf[:, dst_off + c:dst_off + c + 1],
                        scalar2=None, op0=OP.is_equal)
                    nc.tensor.matmul(aggp[:, :ndst], lhsT=wpay[:, c, :],
                                     rhs=oh[:, :ndst], start=(c == 0),
                                     stop=(c == nch - 1))
                    nc.tensor.matmul(zp[:, :ndst], lhsT=w16[:, c:c + 1],
                                     rhs=oh[:, :ndst], start=(c == 0),
                                     stop=(c == nch - 1))
                zsb = pool.tile([1, 512], FT, tag='zsb')
                nc.vector.tensor_copy(zsb[:, :ndst], zp[:, :ndst])
                zr = pool.tile([1, 512], FT, tag='zr')
                nc.vector.reciprocal(zr[:, :ndst], zsb[:, :ndst])
                rbp = pps.tile([128, 512], FT, tag='mm1')
                nc.tensor.matmul(rbp[:, :ndst], lhsT=ones1[:],
                                 rhs=zr[:, :ndst], start=True, stop=True)
                rb = pool.tile([128, 512], FT, tag='rb')
                nc.vector.tensor_copy(rb[:, :ndst], rbp[:, :ndst])
                aggn = pool.tile([128, 512], BF, tag='aggn')
                nc.vector.tensor_mul(aggn[:, :ndst], aggp[:, :ndst],
                                     rb[:, :ndst])
                rp = pps.tile([128, 512], FT, tag='mm1')
                nc.tensor.matmul(rp[:, :ndst], lhsT=gw16[:],
                                 rhs=aggn[:, :ndst], start=True, stop=True)
                rl = pool.tile([128, 512], BF, tag='rl')
                nc.scalar.activation(out=rl[:, :ndst], in_=rp[:, :ndst],
                                     func=AF.Relu, bias=gb_sb[:])
                fdT = pool.tile([128, 512], BF, tag='fdT')
                transpose_rows(fdT, fdrows, JD)
                outT = pool.tile([128, 512], BF, tag='outT')
                if ndr > ndst:
                    nc.vector.memset(outT[:, ndst:ndr], 0.0)
                nc.vector.tensor_add(outT[:, :ndst], fdT[:, :ndst],
                                     rl[:, :ndst])
                return outT

            if PHASE < 6:
                raise _PhaseDone()
            # fire the FTAB gathers (pay0/fde0/fdrows0/curT) now that the
            # collective has written FTAB
            if PREP:
                order_after(FTAB, scr0)
                nc.gpsimd.trigger_dma(count=None, queue_num=0)
            else:
                gat0_gathers()
            cur_gather()
            with (
                tc.tile_pool(name='gat', bufs=1) as gp0,
                tc.tile_pool(name='gat_ps', bufs=1, space='PSUM') as gps,
            ):
                f1T = gat_layer(gp0, gps, pay0, fde0, fdrows0, NCH0, 0,
                                DST0_PER_CORE, gw0_16, gb0_sb)
                a2 = gp0.tile([128, 3, 128], BF)
                transpose_cols(a2, f1T, 3)
                nc.sync.dma_start(cont_ap(AG2IN, 3), a2[:])
                nc.gpsimd.collective_compute(
                    'AllGather', OP.bypass,
                    replica_groups=[list(range(NCORES))],
                    ins=[AG2IN.opt()], outs=[F1TAB.opt()])
                if PREP:
                    order_after(F1TAB, scr1)
                    nc.gpsimd.trigger_dma(count=None, queue_num=1)
                else:
                    gat1_gathers()

                f2T = gat_layer(gp0, gps, pay1, fde1, fdrows1, NCH1,
                                NCH0, DST1_PER_CORE, gw1_16, gb1_sb)
                # f2 rows (64 local dst) -> AllGather -> full [512, D]
                f2rows = gp0.tile([128, 1, 128], BF)
                transpose_cols(f2rows, f2T, 1)
                a3 = AG3IN[:]
                nc.sync.dma_start(
                    bass.AP(tensor=a3.tensor, offset=a3.offset,
                            ap=[[D, DST1_PER_CORE], [1, D]]),
                    f2rows[:DST1_PER_CORE, 0, :])
                nc.gpsimd.collective_compute(
                    'AllGather', OP.bypass,
                    replica_groups=[list(range(NCORES))],
                    ins=[AG3IN.opt()], outs=[F2TAB.opt()])
                f2stg = gp0.tile([128, 4, 128], BF)
                nc.sync.dma_start(f2stg[:], rows_ap(F2TAB, 4))
                f2Tg = glob.tile([128, 512], BF)
                transpose_rows(f2Tg, f2stg, 4)

            sr16 = glob.tile([128, 512], BF)
            with tc.tile_pool(name='sr_ps', bufs=1, space='PSUM') as srps:
                srp = srps.tile([128, 512], FT, tag='srp')
                nc.tensor.matmul(srp[:], lhsT=w2_16[:, 0, :],
                                 rhs=curT[:, 0, :], start=True, stop=False)
                nc.tensor.matmul(srp[:], lhsT=w2_16[:, 1, :], rhs=f2Tg[:],
                                 start=False, stop=True)
                nc.vector.tensor_copy(sr16[:], srp[:])

            # ================= logits =================
            if PHASE < 7:
                raise _PhaseDone()
            with (
                tc.tile_pool(name='lg_o', bufs=4) as lop,
                tc.tile_pool(name='lg_ps', bufs=4, space='PSUM') as lps,
            ):
                for m in range(4):
                    for n in range((LSH + 511) // 512):
                        cs = n * 512
                        cw = min(512, LSH - cs)
                        ps = lps.tile([128, 512], FT, tag='lgps')
                        nc.tensor.matmul(ps[:, :cw],
                                         lhsT=sr16[:, m * 128:(m + 1) * 128],
                                         rhs=itemT[:, cs:cs + cw],
                                         start=True, stop=True)
                        ob = lop.tile([128, 512], F16, tag='ob')
                        if n % 2 == 0:
                            nc.scalar.copy(ob[:, :cw], ps[:, :cw])
                        else:
                            nc.vector.tensor_copy(ob[:, :cw], ps[:, :cw])
                        nc.sync.dma_start(
                            bass.AP(tensor=out_p, offset=m * 128 * LSH + cs,
                                    ap=[[LSH, 128], [1, cw]]),
                            ob[:, :cw])

            ctx.__exit__(None, None, None)
        except _PhaseDone:
            ctx.__exit__(None, None, None)
    nc.compile()
    return nc


_CACHE = {}


def prepare(inputs):
    in_maps, meta = host_prep(inputs)
    import os
    key = (meta['NL'], meta['E0C'], meta['E1C'], tuple(meta['act']),
           os.environ.get('KPHASE', '9'), os.environ.get('KPREP', '0'))
    if key not in _CACHE:
        _CACHE[key] = build_program(meta)
    return _CACHE[key], in_maps, meta


def kernel(**inputs):
    from concourse.bass_utils import run_bass_kernel_spmd
    nc, in_maps, meta = prepare(inputs)
    res = run_bass_kernel_spmd(nc, in_maps, list(range(NCORES)))
    out = np.concatenate([res.results[k]['out'] for k in range(NCORES)],
                         axis=1)
    return np.ascontiguousarray(out.astype(np.float32))


# revision 47
# speedup vs baseline: 139.9301x; 1.2269x over previous
"""DGRec kernel for 8 TRN2 NeuronCores (Bass/Tile).

Strategy (v2):
  - Host: index-only prep + table row-selection sharding. Live-session
    pruning, sessions sorted by lens desc and dealt round-robin so every
    core shares one static shrinking-prefix LSTM schedule. LSTM token
    embeddings are host-expanded per (step, session) into a contiguous
    bf16 buffer (no device gathers for x); renorm happens on device per
    step, fused with a scale-folded PE transpose (matmul vs diag(rcp)).
  - Device: LSTM gates via 4-slot PSUM matmuls, vector bias add + batched
    sigmoid/tanh ACT (gates reordered i,f,o,g so the 3 sigmoids batch),
    bf16 DVE state updates. GAT0 edge-sharded by dst with one-hot
    PE expansion of fd rows (no per-edge fd gather). GAT1 dst-sharded
    8-ways + tiny f2 AllGather. Logits item table renormed + PE-transposed
    fully in SBUF (no DRAM roundtrip/gather). fp16 output, host upcast.
"""
import sys
sys.path.insert(0, '/opt/trn_rl_repo')

import numpy as np
import ml_dtypes

BF16 = ml_dtypes.bfloat16


class _PhaseDone(Exception):
    pass


NCORES = 8
D = 128
T = 20
N0 = 25600
N1 = 2560
N2 = 512
E1 = 5120
NI = 50000
CURB = 128          # cur block rows per core (slot CURB-1 reserved zero row)
DST0_PER_CORE = N1 // NCORES  # 320
DST1_PER_CORE = N2 // NCORES  # 64
B1 = 384            # feat1 block rows per core (320 padded to 384)
ZR1 = 383           # zero row in F1TAB (core 0 block, col 383 of f1T)
CH = 384            # LSTM gate chunk columns
GPERM = [0, 1, 3, 2]  # gate order i,f,o,g (from reference i,f,g,o)


def _rup(x, m):
    return (int(x) + m - 1) // m * m


def _wrap16(idx):
    """[n] int -> [128, n//16] int16: idx i at [i%16, i//16], replicated x8."""
    idx = np.asarray(idx, np.int16)
    n = len(idx)
    assert n % 16 == 0
    a = idx.reshape(n // 16, 16).T  # [16, n//16]
    return np.tile(a, (8, 1))


def _wrap_rows(rows, J):
    """[n<=128J, D] -> [128J, D] cont-ap layout: row (s%128)*J + s//128 = rows[s]."""
    n = len(rows)
    out = np.zeros((128 * J, D), rows.dtype)
    s = np.arange(n)
    out[(s % 128) * J + s // 128] = rows
    return out


def _perm_gates(w):
    """Reorder 4*D gate rows from (i,f,g,o) to (i,f,o,g)."""
    c = [w[g * D:(g + 1) * D] for g in range(4)]
    return np.concatenate([c[g] for g in GPERM], axis=0)


def host_prep(inputs):
    lens = np.asarray(inputs['lens']).astype(np.int64)
    seqs = np.asarray(inputs['padded_seqs']).astype(np.int64)
    uids = np.asarray(inputs['uids']).astype(np.int64)
    cur_sidx = np.asarray(inputs['cur_sidx']).astype(np.int64)
    src0 = np.asarray(inputs['src0']).astype(np.int64)
    dst0 = np.asarray(inputs['dst0']).astype(np.int64)
    idx0 = np.asarray(inputs['idx0']).astype(np.int64)
    src1 = np.asarray(inputs['src1']).astype(np.int64)
    dst1 = np.asarray(inputs['dst1']).astype(np.int64)
    idx1 = np.asarray(inputs['idx1']).astype(np.int64)
    user_emb = np.asarray(inputs['user_emb'], np.float32)
    item_emb = np.ascontiguousarray(
        np.asarray(inputs['item_emb'], np.float32)).astype(BF16)
    item_emb[0] = 0.0  # padding_idx

    # ---- live sessions, sorted by len desc, per-len-group padded to mult 8
    live_mask = np.zeros(N0, bool)
    live_mask[src0] = True
    live_mask[idx0] = True
    live_mask[cur_sidx] = True
    live = np.where(live_mask)[0]
    order = live[np.argsort(-lens[live], kind='stable')]
    lens_live = lens[order]
    parts = []
    grp_ceil = {}
    for L in range(T, 0, -1):
        grp = order[lens_live == L]
        pad = (-len(grp)) % NCORES
        parts.append(grp)
        grp_ceil[L] = (len(grp) + pad) // NCORES
        if pad:
            parts.append(np.full(pad, -1, np.int64))
    order_p = np.concatenate(parts)
    percore = len(order_p) // NCORES
    NL = _rup(percore, 128)
    extra = NL * NCORES - len(order_p)
    order_p = np.concatenate([order_p, np.full(extra, -1, np.int64)])
    core_sessions = [order_p[k::NCORES] for k in range(NCORES)]
    act = [sum(grp_ceil[L] for L in range(t + 1, T + 1)) for t in range(T)]
    act = [min(a, NL) for a in act]

    # ---- FTAB layout: per-core block [NL feat rows][CURB cur rows]
    BLK = NL + CURB
    ZROW = NL + CURB - 1  # core 0 block, slot CURB-1: reserved all-zero row
    sess2pos = np.full(N0, -1, np.int64)
    core_local = []       # per core: {session: local index}
    JU_ = NL // 128
    for k in range(NCORES):
        sess = core_sessions[k]
        real = sess >= 0
        li = np.where(real)[0]
        sess2pos[sess[real]] = k * BLK + (li % 128) * JU_ + li // 128
        core_local.append({int(s): i for i, s in enumerate(sess) if s >= 0})
    # cur block: per core, unique local cur sessions -> slots 0..cnt-1
    cur_pos = {}
    cur_slot_local = [[] for _ in range(NCORES)]
    for s in np.unique(cur_sidx):
        owner = -1
        for k in range(NCORES):
            if int(s) in core_local[k]:
                owner = k
                break
        assert owner >= 0
        slot = len(cur_slot_local[owner])
        assert slot < 112
        cur_pos[int(s)] = owner * BLK + NL + slot
        cur_slot_local[owner].append(core_local[owner][int(s)])
    # remap: sessions in cur_sidx read hn (feat.at[cur].set(cur))
    for s in np.unique(cur_sidx):
        sess2pos[s] = cur_pos[int(s)]

    # ---- per-step x expansion layout
    a128s = [_rup(max(act[t], 1), 128) for t in range(T)]
    xoff = np.cumsum([0] + a128s).tolist()
    SUMA = xoff[-1]

    # ---- GAT0: edges sharded by dst range, sorted by dst
    g0 = []
    for k in range(NCORES):
        lo, hi = k * DST0_PER_CORE, (k + 1) * DST0_PER_CORE
        e = np.where((dst0 >= lo) & (dst0 < hi))[0]
        e = e[np.argsort(dst0[e], kind='stable')]
        g0.append(e)
    E0C = _rup(max(len(e) for e in g0), 128)
    NCH0 = E0C // 128

    # ---- GAT1: edges sharded by dst range (64 per core), sorted by dst
    g1 = []
    for k in range(NCORES):
        lo, hi = k * DST1_PER_CORE, (k + 1) * DST1_PER_CORE
        e = np.where((dst1 >= lo) & (dst1 < hi))[0]
        e = e[np.argsort(dst1[e], kind='stable')]
        g1.append(e)
    E1C = _rup(max(len(e) for e in g1), 128)
    NCH1 = E1C // 128

    def f1pos(node):
        node = np.asarray(node)
        loc = node % DST0_PER_CORE
        return (node // DST0_PER_CORE) * B1 + (loc % 128) * 3 + loc // 128

    cur_idx = np.array([cur_pos[int(s)] for s in cur_sidx], np.int64)

    LSH = NI // NCORES          # 6250
    LSHP = _rup(LSH, 128)       # 6272
    JL = LSHP // 128

    meta = dict(NL=NL, BLK=BLK, E0C=E0C, NCH0=NCH0, E1C=E1C, NCH1=NCH1,
                LSH=LSH, LSHP=LSHP, act=act, ZROW=ZROW, SUMA=SUMA, xoff=xoff)

    # ---- IDX16 buffer layout (columns of 16-wrapped idx); same offsets all cores
    seg_off = {}
    _w = [0]

    def add_seg(name, n):
        seg_off[name] = _w[0]
        _w[0] += n // 16

    add_seg('g0pay', E0C)
    add_seg('g0fd', E0C)
    add_seg('g0res', B1)
    add_seg('g1pay', E1C)
    add_seg('g1fde', E1C)
    add_seg('g1fd', 128)
    add_seg('cur', 512)
    add_seg('hnrow', 112)
    W16 = _w[0]
    meta['seg_off'] = seg_off
    meta['W16'] = W16

    in_maps = []
    for k in range(NCORES):
        sess = core_sessions[k]
        smax = np.maximum(sess, 0)

        # x expansion: transposed [D, SUMA]; col xoff[t]+s = step t, session s
        xseq = np.zeros((D, SUMA), BF16)
        for t in range(T):
            a = act[t]
            if a == 0:
                break
            toks = np.where(sess[:a] >= 0, seqs[smax[:a], t], 0)
            xseq[:, xoff[t]:xoff[t] + a] = item_emb[toks].T

        usr_rows = user_emb[uids[smax]].astype(BF16)
        usr_rows[sess < 0] = 0.0
        usr = _wrap_rows(usr_rows[:NL], NL // 128)

        ishard = _wrap_rows(
            np.asarray(item_emb[1 + k * LSH: 1 + (k + 1) * LSH]), JL)

        idx16 = np.zeros((128, W16), np.int16)

        def put(name, vals):
            v = np.asarray(vals, np.int64)
            assert v.min() >= -1 and v.max() < 32767, (name, v.min(), v.max())
            o = seg_off[name]
            w = _wrap16(v.astype(np.int16))
            idx16[:, o:o + w.shape[1]] = w

        e = g0[k]
        pay = np.full(E0C, ZROW, np.int64)
        fde = np.full(E0C, ZROW, np.int64)
        dstl = np.full(E0C, -1, np.int64)
        pay[:len(e)] = sess2pos[src0[e]]
        fde[:len(e)] = sess2pos[idx0[dst0[e]]]
        dstl[:len(e)] = dst0[e] - k * DST0_PER_CORE
        put('g0pay', pay)
        put('g0fd', fde)
        res0 = np.full(B1, ZROW, np.int64)
        res0[:DST0_PER_CORE] = sess2pos[
            idx0[k * DST0_PER_CORE:(k + 1) * DST0_PER_CORE]]
        put('g0res', res0)

        e1 = g1[k]
        pay1 = np.full(E1C, ZR1, np.int64)
        fde1 = np.full(E1C, ZR1, np.int64)
        dstl1 = np.full(E1C, -1, np.int64)
        pay1[:len(e1)] = f1pos(src1[e1])
        fde1[:len(e1)] = f1pos(idx1[dst1[e1]])
        dstl1[:len(e1)] = dst1[e1] - k * DST1_PER_CORE
        put('g1pay', pay1)
        put('g1fde', fde1)
        fd1 = np.full(128, ZR1, np.int64)
        fd1[:DST1_PER_CORE] = f1pos(
            idx1[k * DST1_PER_CORE:(k + 1) * DST1_PER_CORE])
        put('g1fd', fd1)
        put('cur', cur_idx)
        hnrow = np.zeros(112, np.int64)
        cs = np.asarray(cur_slot_local[k], np.int64)
        assert len(cs) <= 112
        if len(cs):
            hnrow[:len(cs)] = (cs % 128) * (NL // 128) + cs // 128
        put('hnrow', hnrow)

        # dst-local values for one-hot compare: edge (chunk c, partition p)
        dstl_all = np.full((128, NCH0 + NCH1), -1, np.int32)
        dstl_all[:, :NCH0] = dstl.reshape(NCH0, 128).T
        dstl_all[:, NCH0:] = dstl1.reshape(NCH1, 128).T

        in_maps.append({
            'xseq': xseq,
            'usr': usr,
            'ishard': ishard,
            'WihT': np.ascontiguousarray(
                _perm_gates(np.asarray(inputs['Wih'], np.float32)).T
            ).astype(BF16),
            'WhhT': np.ascontiguousarray(
                _perm_gates(np.asarray(inputs['Whh'], np.float32)).T
            ).astype(BF16),
            'bih': _perm_gates(np.asarray(inputs['bih'], np.float32)[:, None])[:, 0],
            'bhh': _perm_gates(np.asarray(inputs['bhh'], np.float32)[:, None])[:, 0],
            'W1T': np.ascontiguousarray(
                np.asarray(inputs['W1'], np.float32).T).astype(BF16),
            'W2T': np.ascontiguousarray(
                np.asarray(inputs['W2'], np.float32).T).astype(BF16),
            'gW0T': np.ascontiguousarray(
                np.asarray(inputs['gW0'], np.float32).T).astype(BF16),
            'gW1T': np.ascontiguousarray(
                np.asarray(inputs['gW1'], np.float32).T).astype(BF16),
            'gb0': np.asarray(inputs['gb0'], np.float32),
            'gb1': np.asarray(inputs['gb1'], np.float32),
            'idx16': idx16,
            'dstl': dstl_all,
        })
    return in_maps, meta


# ============================ device program ============================

def build_program(meta):
    import os
    PHASE = int(os.environ.get('KPHASE', '9'))
    import contextlib
    import concourse.bass as bass
    import concourse.mybir as mybir
    import concourse.tile as tile
    from concourse import bacc
    from concourse.masks import make_identity

    NL = meta['NL']
    BLK = meta['BLK']
    NCH0 = meta['NCH0']
    NCH1 = meta['NCH1']
    LSH = meta['LSH']
    LSHP = meta['LSHP']
    act = meta['act']
    seg = meta['seg_off']
    W16 = meta['W16']
    xoff = meta['xoff']
    JU = NL // 128
    JL = LSHP // 128
    A0 = _rup(act[0], 128)
    J0 = A0 // 128
    FT = mybir.dt.float32
    BF = mybir.dt.bfloat16
    F16 = mybir.dt.float16
    AF = mybir.ActivationFunctionType
    OP = mybir.AluOpType

    nc = bacc.Bacc("TRN2", target_bir_lowering=False, debug=False,
                   num_devices=NCORES, num_swdge_queues=2)

    def param(name, shape, dt=FT):
        return nc.declare_dram_parameter(name, list(shape), dt, isOutput=False)

    xseq_p = param('xseq', [D, meta['SUMA']], BF)
    usr_p = param('usr', [NL, D], BF)
    ishard_p = param('ishard', [LSHP, D], BF)
    WihT = param('WihT', [D, 512], BF)
    WhhT = param('WhhT', [D, 512], BF)
    bih = param('bih', [512])
    bhh = param('bhh', [512])
    W1T = param('W1T', [256, D], BF)
    W2T = param('W2T', [256, D], BF)
    gW0T = param('gW0T', [D, D], BF)
    gW1T = param('gW1T', [D, D], BF)
    gb0 = param('gb0', [D])
    gb1 = param('gb1', [D])
    idx16_p = param('idx16', [128, W16], mybir.dt.int16)
    dstl_p = param('dstl', [128, NCH0 + NCH1], mybir.dt.int32)
    out_p = nc.declare_dram_parameter('out', [N2, LSH], F16, isOutput=True)

    def rows_ap(handle_ap, j_count, base_elem=0):
        """view rows [128*j_count, D] of a DRAM tensor as [128, j, D], row=128j+p"""
        t = handle_ap if isinstance(handle_ap, bass.AP) else handle_ap[:]
        return bass.AP(tensor=t.tensor, offset=t.offset + base_elem,
                       ap=[[D, 128], [128 * D, j_count], [1, D]])

    def cont_ap(handle_ap, j_count, base_elem=0):
        """contiguous [128, j, D] view: slot (p, j) -> DRAM row p*j_count + j"""
        t = handle_ap if isinstance(handle_ap, bass.AP) else handle_ap[:]
        return bass.AP(tensor=t.tensor, offset=t.offset + base_elem,
                       ap=[[j_count * D, 128], [D, j_count], [1, D]])

    def bcast_free(ap, n0, nb):
        """[128, n0] AP -> [128, n0, nb] with 0-stride innermost broadcast."""
        return bass.AP(tensor=ap.tensor, offset=ap.offset,
                       ap=[ap.ap[0], [ap.ap[1][0], n0], [0, nb]])

    with tile.TileContext(nc) as tc:
        try:
            ctx = contextlib.ExitStack()
            ctx.__enter__()
            glob = ctx.enter_context(tc.tile_pool(name='glob', bufs=1))
            dram = ctx.enter_context(tc.tile_pool(name='dram', bufs=1,
                                                  space='DRAM'))
            tps = ctx.enter_context(
                tc.tile_pool(name='tps', bufs=2, space='PSUM'))

            HNROWS = dram.tile([NL, D], BF)
            AGIN = dram.tile([BLK, D], BF)
            FTAB = dram.tile([NCORES * BLK, D], BF, addr_space='Shared')
            AG2IN = dram.tile([B1, D], BF)
            F1TAB = dram.tile([NCORES * B1, D], BF, addr_space='Shared')
            AG3IN = dram.tile([DST1_PER_CORE, D], BF)
            F2TAB = dram.tile([N2, D], BF, addr_space='Shared')

            # ---- global constants / index tiles
            idx_sb = glob.tile([128, W16], mybir.dt.int16)
            nc.sync.dma_start(idx_sb[:], idx16_p[:])

            def seg_ap(name, n, off=0):
                o = seg[name] + off // 16
                return idx_sb[:, o:o + n // 16]

            GMAX = 512

            def gather_t(out_full, colbase, tab, name, n, queue=0):
                """transpose-mode gather of n idx (mult 128) from segment
                `name` into out_full[:, 0, colbase:colbase+n], split <=GMAX"""
                for o in range(0, n, GMAX):
                    w = min(GMAX, n - o)
                    nc.gpsimd.dma_gather(
                        out_ap=out_full[:, :, colbase + o:colbase + o + w],
                        in_ap=tab[:], idxs_ap=seg_ap(name, w, o),
                        num_idxs=w, num_idxs_reg=w, elem_size=D,
                        transpose=True, queue_num=queue)

            def gather_rows(out_tile, tab, name, n, queue=0):
                """non-transpose gather of n idx into [128, n//128, 128]"""
                for o in range(0, n, GMAX):
                    w = min(GMAX, n - o)
                    nc.gpsimd.dma_gather(
                        out_ap=out_tile[:, o // 128:(o + w) // 128, :],
                        in_ap=tab[:], idxs_ap=seg_ap(name, w, o),
                        num_idxs=w, num_idxs_reg=w, elem_size=D,
                        transpose=False, queue_num=queue)

            ident = glob.tile([128, 128], BF)
            make_identity(nc, ident[:])
            ident4 = glob.tile([128, 4, 128], BF)
            for g in range(4):
                nc.vector.tensor_copy(ident4[:, g, :], ident[:])
            ones128 = glob.tile([128, 128], BF)
            nc.vector.memset(ones128[:], 1.0)
            iota_i = glob.tile([128, 512], mybir.dt.int32)
            nc.gpsimd.iota(iota_i[:], pattern=[[1, 512]], base=0,
                           channel_multiplier=0)
            iotaf = glob.tile([128, 512], FT)
            nc.vector.tensor_copy(iotaf[:], iota_i[:])
            dstl_i = glob.tile([128, NCH0 + NCH1], mybir.dt.int32)
            nc.sync.dma_start(dstl_i[:], dstl_p[:])
            dstf = glob.tile([128, NCH0 + NCH1], FT)
            nc.vector.tensor_copy(dstf[:], dstl_i[:])
            ones1 = glob.tile([1, 128], FT)
            nc.vector.memset(ones1[:], 1.0)

            # ---- weights (already bf16 from host)
            wih16 = glob.tile([128, 512], BF)
            nc.sync.dma_start(wih16[:], WihT[:])
            whh16 = glob.tile([128, 512], BF)
            nc.sync.dma_start(whh16[:], WhhT[:])
            w1_16 = glob.tile([128, 2, 128], BF)
            nc.sync.dma_start(w1_16[:], rows_ap(W1T, 2))
            w2_16 = glob.tile([128, 2, 128], BF)
            nc.sync.dma_start(w2_16[:], rows_ap(W2T, 2))
            gw0_16 = glob.tile([128, 128], BF)
            nc.sync.dma_start(gw0_16[:], gW0T[:])
            gw1_16 = glob.tile([128, 128], BF)
            nc.sync.dma_start(gw1_16[:], gW1T[:])
            gb0_sb = glob.tile([128, 1], FT)
            nc.sync.dma_start(gb0_sb[:], bass.AP(tensor=gb0, offset=0,
                                                 ap=[[1, 128], [1, 1]]))
            gb1_sb = glob.tile([128, 1], FT)
            nc.sync.dma_start(gb1_sb[:], bass.AP(tensor=gb1, offset=0,
                                                 ap=[[1, 128], [1, 1]]))
            bi_sb = glob.tile([128, 4], FT)
            nc.sync.dma_start(bi_sb[:], bass.AP(tensor=bih, offset=0,
                                                ap=[[1, 128], [128, 4]]))
            bh_sb = glob.tile([128, 4], FT)
            nc.sync.dma_start(bh_sb[:], bass.AP(tensor=bhh, offset=0,
                                                ap=[[1, 128], [128, 4]]))
            bias = glob.tile([128, 4], FT)
            nc.vector.tensor_add(bias[:], bi_sb[:], bh_sb[:])

            # ---- renorm rows [128, :J, 128] bf16 -> rcp16 [128, JM] bf16
            def renorm_rcp(pool, stg, J, JM, tag=''):
                sq = pool.tile([128, JM, 128], BF, tag='rn_sq' + tag)
                nc.vector.tensor_mul(sq[:, :J, :], stg[:, :J, :],
                                     stg[:, :J, :])
                sumsq = pool.tile([128, JM], FT, tag='rn_ss' + tag)
                nc.vector.tensor_reduce(out=sumsq[:, :J], in_=sq[:, :J, :],
                                        axis=mybir.AxisListType.X, op=OP.add)
                nrm = pool.tile([128, JM], FT, tag='rn_nrm' + tag)
                nc.scalar.activation(out=nrm[:, :J], in_=sumsq[:, :J],
                                     func=AF.Sqrt)
                nc.vector.tensor_scalar_max(nrm[:, :J], nrm[:, :J], 1e-12)
                rcp = pool.tile([128, JM], FT, tag='rn_rcp' + tag)
                nc.vector.reciprocal(rcp[:, :J], nrm[:, :J])
                nc.vector.tensor_scalar_min(rcp[:, :J], rcp[:, :J], 1.0)
                rcp16 = pool.tile([128, JM], BF, tag='rn_r16' + tag)
                nc.vector.tensor_copy(rcp16[:, :J], rcp[:, :J])
                return rcp16

            # ---- scaled transpose: out[:, 128j+p] = stg[p, j, :] * rcp[p, j]
            def scaled_transpose(pool, outT, stg, rcp16, J, tag=''):
                for g0i in range(0, J, 4):
                    ng = min(4, J - g0i)
                    diag = pool.tile([128, 4, 128], BF, tag='diag' + tag)
                    r = rcp16[:]
                    nc.vector.tensor_tensor(
                        out=diag[:, :ng, :], in0=ident4[:, :ng, :],
                        in1=bass.AP(tensor=r.tensor,
                                    offset=r.offset + g0i * r.ap[1][0],
                                    ap=[r.ap[0], [r.ap[1][0], ng], [0, 128]]),
                        op=OP.mult)
                    tp = tps.tile([128, 4, 128], FT, tag='tp')
                    for j in range(ng):
                        nc.tensor.matmul(tp[:, j, :], lhsT=stg[:, g0i + j, :],
                                         rhs=diag[:, j, :], start=True,
                                         stop=True)
                    nc.vector.tensor_copy(
                        outT[:, (g0i) * 128:(g0i + ng) * 128],
                        tp[:, :ng, :])

            # ---- plain transpose rows->cols: outT[:, 128j+p] = rows[p, j, :]
            def transpose_rows(outT, rows_t, J, cols=None):
                for g0i in range(0, J, 4):
                    ng = min(4, J - g0i)
                    tp = tps.tile([128, 4, 128], FT, tag='tp')
                    for j in range(ng):
                        nc.tensor.matmul(tp[:, j, :],
                                         lhsT=rows_t[:, g0i + j, :],
                                         rhs=ident[:], start=True, stop=True)
                    nc.vector.tensor_copy(
                        outT[:, g0i * 128:(g0i + ng) * 128], tp[:, :ng, :])

            # ---- transpose cols->rows: rows[p, j, :] = srcT[:, 128j+p]
            def transpose_cols(rows_t, srcT, J):
                for g0i in range(0, J, 4):
                    ng = min(4, J - g0i)
                    tp = tps.tile([128, 4, 128], FT, tag='tp')
                    for j in range(ng):
                        nc.tensor.matmul(
                            tp[:, j, :],
                            lhsT=srcT[:, (g0i + j) * 128:(g0i + j + 1) * 128],
                            rhs=ident[:], start=True, stop=True)
                    nc.vector.tensor_copy(rows_t[:, g0i:g0i + ng, :],
                                          tp[:, :ng, :])

            # ================= user renorm -> longT =================
            hT = glob.tile([128, NL], BF)
            cT = glob.tile([128, NL], BF)
            longT = glob.tile([128, NL], BF)
            nc.vector.memset(hT[:], 0.0)
            nc.vector.memset(cT[:], 0.0)

            with tc.tile_pool(name='upre', bufs=1) as up:
                ustg = up.tile([128, JU, 128], BF)
                nc.sync.dma_start(ustg[:], cont_ap(usr_p, JU))
                urcp = renorm_rcp(up, ustg, JU, JU, tag='u')
                scaled_transpose(up, longT, ustg, urcp, JU, tag='u')

            # xhat buffer: renorm of xseq done per-step, interleaved with LSTM
            if PHASE < 2:
                raise _PhaseDone()
            SUMA = meta['SUMA']
            xhat = glob.tile([128, SUMA], BF)
            xs = xseq_p[:]
            epsb = glob.tile([128, 1], FT)
            nc.vector.memset(epsb[:], 1e-24)
            bias_row = glob.tile([1, 512], BF)
            bi_r = glob.tile([1, 512], FT)
            bh_r = glob.tile([1, 512], FT)
            nc.sync.dma_start(bi_r[:], bass.AP(tensor=bih, offset=0,
                                               ap=[[1, 1], [1, 512]]))
            nc.sync.dma_start(bh_r[:], bass.AP(tensor=bhh, offset=0,
                                               ap=[[1, 1], [1, 512]]))
            nc.vector.tensor_add(bias_row[:], bi_r[:], bh_r[:])
            ones_row = glob.tile([1, 512], BF)
            nc.vector.memset(ones_row[:], 1.0)

            # ---- upfront renorm of the whole xseq (keeps Sqrt table
            # loads contiguous; gpsimd/vector/scalar split)
            with tc.tile_pool(name='xnorm', bufs=4) as xn:
                for cs in range(0, SUMA, 512):
                    cw = min(512, SUMA - cs)
                    xraw = xn.tile([128, 512], BF, tag='xraw')
                    nc.sync.dma_start(
                        xraw[:, :cw],
                        bass.AP(tensor=xs.tensor, offset=xs.offset + cs,
                                ap=[[SUMA, 128], [1, cw]]))
                    sq = xn.tile([128, 512], BF, tag='xsq')
                    nc.vector.tensor_mul(sq[:, :cw], xraw[:, :cw],
                                         xraw[:, :cw])
                    nps = tps.tile([128, 4, 128], FT, tag='tp')
                    n_ = nps[:]
                    psf = bass.AP(tensor=n_.tensor, offset=n_.offset,
                                  ap=[n_.ap[0], [1, cw]])
                    nc.tensor.matmul(psf, lhsT=ones128[:], rhs=sq[:, :cw],
                                     start=True, stop=True)
                    sr_ = xn.tile([128, 512], FT, tag='sr_')
                    nc.scalar.activation(out=sr_[:, :cw], in_=psf,
                                         func=AF.Sqrt, bias=epsb[:])
                    rs = xn.tile([128, 512], FT, tag='rs')
                    nc.vector.reciprocal_approx_fast(rs[:, :cw], sr_[:, :cw])
                    with nc.allow_low_precision(reason='bf16 renorm'):
                        nc.vector.scalar_tensor_tensor(
                            out=xhat[:, cs:cs + cw],
                            in0=rs[:, :cw], scalar=1.0, in1=xraw[:, :cw],
                            op0=OP.min, op1=OP.mult)

            # ========== SWDGE preps: desc-gen overlaps the LSTM ==========
            # (prepare_only path kept for experiments; broken on this
            # runtime — deferred-completion waits misfire -> NaNs)
            PREP = int(os.environ.get('KPREP', '0'))
            _semn = [0]

            def prep_rows(out_tile, tab, name, n, queue):
                for o in range(0, n, GMAX):
                    w = min(GMAX, n - o)
                    sem = None
                    if PREP:
                        sem = nc.alloc_semaphore(f'swdge{_semn[0]}')
                        _semn[0] += 1
                    nc.gpsimd.dma_gather(
                        out_ap=out_tile[:, o // 128:(o + w) // 128, :],
                        in_ap=tab[:], idxs_ap=seg_ap(name, w, o),
                        num_idxs=w, num_idxs_reg=w, elem_size=D,
                        transpose=False, queue_num=queue,
                        prepare_only=bool(PREP), sem=sem)

            pay0 = glob.tile([128, NCH0, 128], BF)
            fde0 = glob.tile([128, NCH0, 128], BF)
            fdrows0 = glob.tile([128, 3, 128], BF)
            curT = glob.tile([128, 1, 512], BF)
            pay1 = glob.tile([128, NCH1, 128], BF)
            fde1 = glob.tile([128, NCH1, 128], BF)
            fdrows1 = glob.tile([128, 1, 128], BF)

            def gat0_gathers():
                prep_rows(pay0, FTAB, 'g0pay', NCH0 * 128, 0)
                prep_rows(fde0, FTAB, 'g0fd', NCH0 * 128, 0)
                prep_rows(fdrows0, FTAB, 'g0res', B1, 0)

            def cur_gather():
                nc.gpsimd.dma_gather(
                    out_ap=curT[:], in_ap=FTAB[:], idxs_ap=seg_ap('cur', 512),
                    num_idxs=512, num_idxs_reg=512, elem_size=D,
                    transpose=True, queue_num=0)

            def gat1_gathers():
                prep_rows(pay1, F1TAB, 'g1pay', NCH1 * 128, 1)
                prep_rows(fde1, F1TAB, 'g1fde', NCH1 * 128, 1)
                prep_rows(fdrows1, F1TAB, 'g1fd', 128, 1)

            if PREP:
                gat0_gathers()
                gat1_gathers()
            # read-back scratch: orders each trigger after its collective
            # (the collective inst is async; RAW via a 1-row DMA + a gpsimd
            # op in front of the trigger on the same FIFO)
            scr0 = glob.tile([1, D], BF)
            scr1 = glob.tile([1, D], BF)
            scrd = glob.tile([1, D], BF)

            def order_after(tab, scr):
                t_ = tab[:]
                nc.sync.dma_start(
                    scr[:], bass.AP(tensor=t_.tensor, offset=t_.offset,
                                    ap=[[D, 1], [1, D]]))
                nc.gpsimd.tensor_copy(scrd[:], scr[:])

            # ================= LSTM (renorm interleaved) =================
            if PHASE < 3:
                raise _PhaseDone()
            with (
                tc.tile_pool(name='lstm_g', bufs=3) as sp,
                tc.tile_pool(name='lstm_ps', bufs=2, space='PSUM') as gp,
            ):
                for t in range(T):
                    a = act[t]
                    if a == 0:
                        break
                    xo = xoff[t]
                    nch = (a + CH - 1) // CH
                    for c in range(nch):
                        cs = c * CH
                        cw = min(CH, a - cs)
                        ce = cs + cw
                        g4 = gp.tile([128, 4, CH], FT, tag='g4')
                        for g in range(4):
                            nc.tensor.matmul(
                                g4[:, g, :cw],
                                lhsT=wih16[:, g * 128:(g + 1) * 128],
                                rhs=xhat[:, xo + cs:xo + ce],
                                start=True, stop=(t == 0))
                            if t > 0:
                                nc.tensor.matmul(
                                    g4[:, g, :cw],
                                    lhsT=whh16[:, g * 128:(g + 1) * 128],
                                    rhs=hT[:, cs:ce], start=False, stop=True)
                        gb = sp.tile([128, 4, CH], BF, tag='gb')
                        nc.vector.tensor_tensor(
                            out=gb[:, :, :cw], in0=g4[:, :, :cw],
                            in1=bcast_free(bias[:], 4, cw), op=OP.add)
                        sg = sp.tile([128, 4, CH], BF, tag='sg')
                        nc.scalar.activation(out=sg[:, :3, :cw],
                                             in_=gb[:, :3, :cw],
                                             func=AF.Sigmoid)
                        nc.scalar.activation(out=sg[:, 3, :cw],
                                             in_=gb[:, 3, :cw], func=AF.Tanh)
                        # state update: i=0 f=1 o=2 g=3
                        if t > 0:
                            tmp = sp.tile([128, CH], BF, tag='tmp')
                            nc.vector.tensor_mul(tmp[:, :cw], sg[:, 0, :cw],
                                                 sg[:, 3, :cw])
                            nc.vector.tensor_mul(cT[:, cs:ce], cT[:, cs:ce],
                                                 sg[:, 1, :cw])
                            nc.vector.tensor_add(cT[:, cs:ce], cT[:, cs:ce],
                                                 tmp[:, :cw])
                        else:
                            nc.vector.tensor_mul(cT[:, cs:ce], sg[:, 0, :cw],
                                                 sg[:, 3, :cw])
                        th = sp.tile([128, CH], BF, tag='th')
                        nc.scalar.activation(out=th[:, :cw], in_=cT[:, cs:ce],
                                             func=AF.Tanh)
                        nc.vector.tensor_mul(hT[:, cs:ce], sg[:, 2, :cw],
                                             th[:, :cw])

            # ============ feat + transposes + AG1 ============
            if PHASE < 4:
                raise _PhaseDone()
            with (
                tc.tile_pool(name='feat', bufs=1) as fp,
                tc.tile_pool(name='feat_ps', bufs=2, space='PSUM') as fps,
            ):
                featT = fp.tile([128, NL], BF)
                for c in range((NL + 511) // 512):
                    cs = c * 512
                    cw = min(512, NL - cs)
                    ps = fps.tile([128, 512], FT, tag='fps')
                    nc.tensor.matmul(ps[:, :cw], lhsT=w1_16[:, 0, :],
                                     rhs=longT[:, cs:cs + cw], start=True,
                                     stop=False)
                    nc.tensor.matmul(ps[:, :cw], lhsT=w1_16[:, 1, :],
                                     rhs=hT[:, cs:cs + cw], start=False,
                                     stop=True)
                    nc.scalar.activation(out=featT[:, cs:cs + cw],
                                         in_=ps[:, :cw], func=AF.Relu)

                fr = fp.tile([128, JU, 128], BF)
                hr = fp.tile([128, JU, 128], BF)
                transpose_cols(fr, featT, JU)
                transpose_cols(hr, hT, JU)
                nc.sync.dma_start(cont_ap(HNROWS, JU), hr[:])
                nc.sync.dma_start(cont_ap(AGIN, JU), fr[:])
                curs = fp.tile([128, 1, 128], BF)
                nc.vector.memset(curs[:], 0.0)
                nc.gpsimd.dma_gather(
                    out_ap=curs[:], in_ap=HNROWS[:],
                    idxs_ap=seg_ap('hnrow', 112),
                    num_idxs=112, num_idxs_reg=112, elem_size=D,
                    transpose=False, queue_num=0)
                ag = AGIN[:]
                nc.sync.dma_start(
                    bass.AP(tensor=ag.tensor, offset=ag.offset + NL * D,
                            ap=[[D, 128], [1, D]]),
                    curs[:, 0, :])
                nc.gpsimd.collective_compute(
                    'AllGather', OP.bypass,
                    replica_groups=[list(range(NCORES))],
                    ins=[AGIN.opt()], outs=[FTAB.opt()])

            # ============ logits table prep (overlaps collective) ============
            if PHASE < 5:
                raise _PhaseDone()
            itemT = glob.tile([128, LSHP], BF)
            with tc.tile_pool(name='lg_i', bufs=1) as lp:
                ls = lp.tile([128, JL, 128], BF)
                nc.sync.dma_start(ls[:], cont_ap(ishard_p, JL))
                lrcp = renorm_rcp(lp, ls, JL, JL, tag='l')
                scaled_transpose(lp, itemT, ls, lrcp, JL, tag='l')

            # ================= GAT layers =================
            def gat_layer(pool, pps, pay, fde, fdrows, nch, dst_off,
                          ndst, gw16, gb_sb):
                """Returns outT [128, rup(ndst,128)] = fd + relu(agg @ W + b)."""
                ndr = _rup(ndst, 128)
                JD = ndr // 128
                score = pool.tile([128, nch], FT, tag='score')
                prod = pool.tile([128, nch, 128], BF, tag='prod')
                nc.vector.tensor_mul(prod[:], pay[:], fde[:])
                nc.vector.tensor_reduce(out=score[:], in_=prod[:],
                                        axis=mybir.AxisListType.X, op=OP.add)
                w = pool.tile([128, nch], FT, tag='w')
                nc.scalar.activation(out=w[:], in_=score[:], func=AF.Exp)
                w16 = pool.tile([128, nch], BF, tag='w16')
                nc.vector.tensor_copy(w16[:], w[:])
                wpay = pool.tile([128, nch, 128], BF, tag='wpay')
                wv = w16[:]
                nc.vector.tensor_tensor(out=wpay[:], in0=pay[:],
                                        in1=bcast_free(wv, nch, 128),
                                        op=OP.mult)
                aggp = pps.tile([128, 512], FT, tag='aggp')
                zp = pps.tile([1, 512], FT, tag='zp')
                for c in range(nch):
                    oh = pool.tile([128, 512], BF, tag='oh', bufs=2)
                    nc.vector.tensor_scalar(
                        out=oh[:, :ndst], in0=iotaf[:, :ndst],
                        scalar1=dstf[:, dst_off + c:dst_off + c + 1],
                        scalar2=None, op0=OP.is_equal)
                    nc.tensor.matmul(aggp[:, :ndst], lhsT=wpay[:, c, :],
                                     rhs=oh[:, :ndst], start=(c == 0),
                                     stop=(c == nch - 1))
                    nc.tensor.matmul(zp[:, :ndst], lhsT=w16[:, c:c + 1],
                                     rhs=oh[:, :ndst], start=(c == 0),
                                     stop=(c == nch - 1))
                zsb = pool.tile([1, 512], FT, tag='zsb')
                nc.vector.tensor_copy(zsb[:, :ndst], zp[:, :ndst])
                zr = pool.tile([1, 512], FT, tag='zr')
                nc.vector.reciprocal(zr[:, :ndst], zsb[:, :ndst])
                rbp = pps.tile([128, 512], FT, tag='mm1')
                nc.tensor.matmul(rbp[:, :ndst], lhsT=ones1[:],
                                 rhs=zr[:, :ndst], start=True, stop=True)
                rb = pool.tile([128, 512], FT, tag='rb')
                nc.vector.tensor_copy(rb[:, :ndst], rbp[:, :ndst])
                aggn = pool.tile([128, 512], BF, tag='aggn')
                nc.vector.tensor_mul(aggn[:, :ndst], aggp[:, :ndst],
                                     rb[:, :ndst])
                rp = pps.tile([128, 512], FT, tag='mm1')
                nc.tensor.matmul(rp[:, :ndst], lhsT=gw16[:],
                                 rhs=aggn[:, :ndst], start=True, stop=True)
                rl = pool.tile([128, 512], BF, tag='rl')
                nc.scalar.activation(out=rl[:, :ndst], in_=rp[:, :ndst],
                                     func=AF.Relu, bias=gb_sb[:])
                fdT = pool.tile([128, 512], BF, tag='fdT')
                transpose_rows(fdT, fdrows, JD)
                outT = pool.tile([128, 512], BF, tag='outT')
                if ndr > ndst:
                    nc.vector.memset(outT[:, ndst:ndr], 0.0)
                nc.vector.tensor_add(outT[:, :ndst], fdT[:, :ndst],
                                     rl[:, :ndst])
                return outT

            if PHASE < 6:
                raise _PhaseDone()
            # fire the FTAB gathers (pay0/fde0/fdrows0/curT) now that the
            # collective has written FTAB
            if PREP:
                order_after(FTAB, scr0)
                nc.gpsimd.trigger_dma(count=None, queue_num=0)
            else:
                gat0_gathers()
            cur_gather()
            with (
                tc.tile_pool(name='gat', bufs=1) as gp0,
                tc.tile_pool(name='gat_ps', bufs=1, space='PSUM') as gps,
            ):
                f1T = gat_layer(gp0, gps, pay0, fde0, fdrows0, NCH0, 0,
                                DST0_PER_CORE, gw0_16, gb0_sb)
                a2 = gp0.tile([128, 3, 128], BF)
                transpose_cols(a2, f1T, 3)
                nc.sync.dma_start(cont_ap(AG2IN, 3), a2[:])
                nc.gpsimd.collective_compute(
                    'AllGather', OP.bypass,
                    replica_groups=[list(range(NCORES))],
                    ins=[AG2IN.opt()], outs=[F1TAB.opt()])
                if PREP:
                    order_after(F1TAB, scr1)
                    nc.gpsimd.trigger_dma(count=None, queue_num=1)
                else:
                    gat1_gathers()

                f2T = gat_layer(gp0, gps, pay1, fde1, fdrows1, NCH1,
                                NCH0, DST1_PER_CORE, gw1_16, gb1_sb)
                # f2 rows (64 local dst) -> AllGather -> full [512, D]
                f2rows = gp0.tile([128, 1, 128], BF)
                transpose_cols(f2rows, f2T, 1)
                a3 = AG3IN[:]
                nc.sync.dma_start(
                    bass.AP(tensor=a3.tensor, offset=a3.offset,
                            ap=[[D, DST1_PER_CORE], [1, D]]),
                    f2rows[:DST1_PER_CORE, 0, :])
                nc.gpsimd.collective_compute(
                    'AllGather', OP.bypass,
                    replica_groups=[list(range(NCORES))],
                    ins=[AG3IN.opt()], outs=[F2TAB.opt()])
                f2stg = gp0.tile([128, 4, 128], BF)
                nc.sync.dma_start(f2stg[:], rows_ap(F2TAB, 4))
                f2Tg = glob.tile([128, 512], BF)
                transpose_rows(f2Tg, f2stg, 4)

            sr16 = glob.tile([128, 512], BF)
            with tc.tile_pool(name='sr_ps', bufs=1, space='PSUM') as srps:
                srp = srps.tile([128, 512], FT, tag='srp')
                nc.tensor.matmul(srp[:], lhsT=w2_16[:, 0, :],
                                 rhs=curT[:, 0, :], start=True, stop=False)
                nc.tensor.matmul(srp[:], lhsT=w2_16[:, 1, :], rhs=f2Tg[:],
                                 start=False, stop=True)
                nc.vector.tensor_copy(sr16[:], srp[:])

            # ================= logits =================
            if PHASE < 7:
                raise _PhaseDone()
            with (
                tc.tile_pool(name='lg_o', bufs=4) as lop,
                tc.tile_pool(name='lg_ps', bufs=4, space='PSUM') as lps,
            ):
                for m in range(4):
                    for n in range((LSH + 511) // 512):
                        cs = n * 512
                        cw = min(512, LSH - cs)
                        ps = lps.tile([128, 512], FT, tag='lgps')
                        nc.tensor.matmul(ps[:, :cw],
                                         lhsT=sr16[:, m * 128:(m + 1) * 128],
                                         rhs=itemT[:, cs:cs + cw],
                                         start=True, stop=True)
                        ob = lop.tile([128, 512], F16, tag='ob')
                        if n % 2 == 0:
                            nc.scalar.copy(ob[:, :cw], ps[:, :cw])
                        else:
                            nc.vector.tensor_copy(ob[:, :cw], ps[:, :cw])
                        nc.sync.dma_start(
                            bass.AP(tensor=out_p, offset=m * 128 * LSH + cs,
                                    ap=[[LSH, 128], [1, cw]]),
                            ob[:, :cw])

            ctx.__exit__(None, None, None)
        except _PhaseDone:
            ctx.__exit__(None, None, None)
    nc.compile()
    return nc


_CACHE = {}


def prepare(inputs):
    in_maps, meta = host_prep(inputs)
    import os
    key = (meta['NL'], meta['E0C'], meta['E1C'], tuple(meta['act']),
           os.environ.get('KPHASE', '9'), os.environ.get('KPREP', '0'))
    if key not in _CACHE:
        _CACHE[key] = build_program(meta)
    return _CACHE[key], in_maps, meta


def kernel(**inputs):
    from concourse.bass_utils import run_bass_kernel_spmd
    nc, in_maps, meta = prepare(inputs)
    res = run_bass_kernel_spmd(nc, in_maps, list(range(NCORES)))
    out = np.concatenate([res.results[k]['out'] for k in range(NCORES)],
                         axis=1)
    return np.ascontiguousarray(out.astype(np.float32))
